# revision 8
# baseline (speedup 1.0000x reference)
"""Trainium2 Bass kernel for 16-head MultiHeadAttention (B=2, S=2048, D=1024, f32).

Sharding: 8 cores = 2 (batch) x 4 (head groups of 4 heads).
Each core gets col-shards of Wq/Wk/Wv + a row-shard of Wo, computes a full
[2048,1024] partial output; the host sums the 8 partials (4 per batch) into
[2,2048,1024].

All device data is bf16 (f32 PSUM accumulation); output bf16 (rel-err budget
2e-2 >> bf16 noise ~6e-3; fp8 was measured at 7e-2 - attention's weighted
average shrinks signal and noise together, so fp8 noise does NOT average out).

The ACT engine (exp over 4 x 2048 x 2048 scores: 128 instrs x ~1.15us =
147us) is the pacing engine; the PE (~155us) runs just behind it:
  - single PSUM pool, two tag rings: "sc" (3 x 2 banks, score tiles) and
    "acc" (2 x 1 bank, every other accumulator) - no pool-boundary barrier;
  - K/Q head-pair-0 projections run before the chunk loop; K-p1 is spread
    through chunk 0's windows, the V projection through chunk 1's, Q-p1
    lazily per chunk - all sized to hide under the ACT windows;
  - scores: two heads row-packed in the PE via tile_position;
  - AV: V_aug ([j, 65], 65th col = ones) stationary, expT moving ->
    outT[65, q] with softmax denominators in row 64 (N=512 streams keep
    the stationary reload hidden);
  - normalize: reciprocal of the denominator row + gpsimd partition
    broadcast + multiply into outT (bf16);
  - Wo: K=256 accumulation over both head pairs -> single bf16 output;
  - xv shares SBUF slots with xk (dead after K-projs); the exp ring (34
    tiles) is deep enough that chunk-1 exps never wait on chunk-0's AV.
"""

import sys

import numpy as np

if "/opt/trn_rl_repo" not in sys.path:
    sys.path.insert(0, "/opt/trn_rl_repo")

import ml_dtypes

import concourse.bacc as bacc
import concourse.mybir as mybir
import concourse.tile as tile

F32 = mybir.dt.float32
BF16 = mybir.dt.bfloat16

B, S, D, H = 2, 2048, 1024, 16
DK = D // H          # 64
HL = 4               # heads per core
DG = HL * DK         # 256
SCALE = 0.125        # 1/sqrt(DK)

ET = D // 128        # 8 e-tiles (contraction over D)
JT = S // 128        # 16 j-tiles (keys)
QC = S // 512        # 4 q-chunks


def _build_nc():
    nc = bacc.Bacc("TRN2", target_bir_lowering=False, debug=False)

    xq = nc.dram_tensor("xq", [D, S], BF16, kind="ExternalInput").ap()
    xk = nc.dram_tensor("xk", [D, S], BF16, kind="ExternalInput").ap()
    xv = nc.dram_tensor("xv", [D, S], BF16, kind="ExternalInput").ap()
    wq = nc.dram_tensor("wq", [D, DG], BF16, kind="ExternalInput").ap()
    wk = nc.dram_tensor("wk", [D, DG], BF16, kind="ExternalInput").ap()
    wv = nc.dram_tensor("wv", [D, DG], BF16, kind="ExternalInput").ap()
    wo = nc.dram_tensor("wo", [DG, D], BF16, kind="ExternalInput").ap()
    out = nc.dram_tensor("out", [S, D], BF16, kind="ExternalOutput").ap()

    with tile.TileContext(nc) as tc:
        with (
            tc.tile_pool(name="wpool", bufs=1) as wpool,
            tc.tile_pool(name="xin", bufs=1) as xin,
            tc.tile_pool(name="proj", bufs=1) as proj,
            tc.tile_pool(name="expp", bufs=34) as expp,
            tc.tile_pool(name="nrm", bufs=4) as nrm,
            tc.tile_pool(name="osbp", bufs=2) as osbp,
            tc.tile_pool(name="ps", bufs=1, space="PSUM") as ps,
        ):
            # ---- ACT warmup: force the Exp table load at t=0 --------------
            wu_in = wpool.tile([128, 16], F32, tag="wu", name="wu_in")
            nc.vector.memset(wu_in, 0.0)
            wu_out = wpool.tile([128, 16], BF16, tag="wuo", name="wu_out")
            nc.scalar.activation(
                out=wu_out, in_=wu_in,
                func=mybir.ActivationFunctionType.Exp, scale=1.0,
            )

            # ---- weight tiles ---------------------------------------------
            wk_sb = [wpool.tile([128, DG], BF16, tag=f"wk{e}", name=f"wk{e}")
                     for e in range(ET)]
            wq_sb = [wpool.tile([128, DG], BF16, tag=f"wq{e}", name=f"wq{e}")
                     for e in range(ET)]
            wv_sb = [wpool.tile([128, DG], BF16, tag=f"wv{e}", name=f"wv{e}")
                     for e in range(ET)]
            wo_sb = [wpool.tile([128, D], BF16, tag=f"wo{p}", name=f"wo{p}")
                     for p in range(2)]

            # ---- persistent activation tiles ------------------------------
            kt_sb = [proj.tile([128, S], BF16, tag=f"kt{p}", name=f"kt{p}")
                     for p in range(2)]
            qt_sb = [proj.tile([128, S], BF16, tag=f"qt{p}", name=f"qt{p}")
                     for p in range(2)]
            v_sb = proj.tile([128, JT, HL, DK + 1], BF16, tag="v", name="v_sb")
            nc.vector.memset(v_sb[:, :, :, DK:DK + 1], 1.0)
            outt_sb = [proj.tile([128, S], BF16, tag=f"ot{p}", name=f"outt{p}")
                       for p in range(2)]

            # ---- DMA emission (sync+gpsimd round-robin) -------------------
            queues = [nc.sync, nc.gpsimd]
            rr = [0]

            def dq(dst, src):
                queues[rr[0] % 2].dma_start(dst, src)
                rr[0] += 1

            # xv reuses xk's slots (xk dead once K-p0 + K-p1 are done, and
            # K-p1 runs inside chunk 0 - before xv's transfers queue up).
            xk_t = [xin.tile([128, S], BF16, tag=f"xkv{e}", name=f"xk{e}")
                    for e in range(ET)]
            xq_t = [xin.tile([128, S], BF16, tag=f"xq{e}", name=f"xq{e}")
                    for e in range(ET)]
            xv_t = [xin.tile([128, S], BF16, tag=f"xkv{e}", name=f"xv{e}")
                    for e in range(ET)]

            def dx(ts, dram, c):
                sl = slice(c * 512, (c + 1) * 512)
                for e in range(ET):
                    dq(ts[e][:, sl], dram[e * 128:(e + 1) * 128, sl])

            def dw(ts, dram):
                for e in range(ET):
                    dq(ts[e], dram[e * 128:(e + 1) * 128, :])

            dw(wk_sb, wk)
            dx(xk_t, xk, 0)
            dw(wq_sb, wq)
            dx(xq_t, xq, 0)
            dx(xk_t, xk, 1)
            dx(xk_t, xk, 2)
            dx(xk_t, xk, 3)
            dx(xq_t, xq, 1)
            dw(wv_sb, wv)
            dx(xv_t, xv, 0)
            dx(xv_t, xv, 1)
            dx(xq_t, xq, 2)
            dx(xv_t, xv, 2)
            dx(xv_t, xv, 3)
            dx(xq_t, xq, 3)
            for p in range(2):
                dq(wo_sb[p], wo[p * 128:(p + 1) * 128, :])

            # ---- projection emitters (all accs ride the "acc" ring) -------
            def kq_proj(p, cs, which):
                for c in cs:
                    csl = slice(c * 512, (c + 1) * 512)
                    srcs = {"k": (wk_sb, xk_t, kt_sb), "q": (wq_sb, xq_t, qt_sb)}
                    for nm in which:
                        w_sb, x_t, dst = srcs[nm]
                        acc = ps.tile([128, 512], F32, tag="acc",
                                      name=f"a{nm}{p}{c}")
                        for e in range(ET):
                            nc.tensor.matmul(
                                acc, w_sb[e][:, p * 128:(p + 1) * 128],
                                x_t[e][:, csl],
                                start=(e == 0), stop=(e == ET - 1))
                        nc.vector.tensor_copy(dst[p][:, csl], acc)

            def v_proj(jts):
                for jt in jts:
                    jsl = slice(jt * 128, (jt + 1) * 128)
                    accv = ps.tile([128, DG], F32, tag="acc", name=f"pv{jt}")
                    for e in range(ET):
                        nc.tensor.matmul(
                            accv, xv_t[e][:, jsl], wv_sb[e],
                            start=(e == 0), stop=(e == ET - 1))
                    nc.vector.tensor_copy(
                        v_sb[:, jt, :, 0:DK],
                        accv.rearrange("j (h d) -> j h d", h=HL))

            # ---- attention tail: AV + normalize (+ Wo) --------------------
            def tail(p, c, ex_tiles):
                csl = slice(c * 512, (c + 1) * 512)
                for h in range(2):
                    hg = 2 * p + h
                    av = ps.tile([128, 512], F32, tag="acc",
                                 name=f"av{p}{c}{h}")
                    for jt in range(JT):
                        nc.tensor.matmul(
                            av[0:DK + 1, :], v_sb[:, jt, hg, :],
                            ex_tiles[jt][:, h, :],
                            start=(jt == 0), stop=(jt == JT - 1))
                    rrow = nrm.tile([1, 512], F32, tag="rr", name=f"rr{p}{c}{h}")
                    nc.vector.reciprocal(rrow, av[DK:DK + 1, :])
                    rbc = nrm.tile([64, 512], F32, tag="rbc", name=f"rb{p}{c}{h}")
                    nc.gpsimd.partition_broadcast(rbc, rrow)
                    nc.vector.tensor_tensor(
                        outt_sb[p][h * 64:(h + 1) * 64, csl],
                        av[0:DK, :], rbc, mybir.AluOpType.mult)
                if p == 1:
                    for k in range(4):
                        qg = c * 4 + k
                        osb = osbp.tile([128, D], BF16, tag="osb",
                                        name=f"osb{c}{k}")
                        for ch in range(2):
                            chsl = slice(ch * 512, (ch + 1) * 512)
                            acc = ps.tile([128, 512], F32, tag="acc",
                                          name=f"po{c}{k}{ch}")
                            nc.tensor.matmul(
                                acc, outt_sb[0][:, qg * 128:(qg + 1) * 128],
                                wo_sb[0][:, chsl], start=True, stop=False)
                            nc.tensor.matmul(
                                acc, outt_sb[1][:, qg * 128:(qg + 1) * 128],
                                wo_sb[1][:, chsl], start=False, stop=True)
                            nc.vector.tensor_copy(osb[:, chsl], acc)
                        nc.sync.dma_start(
                            out[qg * 128:(qg + 1) * 128, :], osb)

            # ---- main flow ------------------------------------------------
            # K-p0 first, then Q-p0-c0/c1, K-p1 (early ring slots so xv's
            # slot-reuse unblocks), then Q-p0-c2/c3 (their xq arrives late).
            kq_proj(0, range(QC), "k")
            kq_proj(0, [0, 1], "q")
            pend_kp1 = list(range(QC))        # K-p1 accs, spread over chunk 0
            pend_q0 = [2, 3]                  # late Q-p0 accs

            pending = None
            for i in range(8):
                p, c = i // 4, i % 4
                if i >= 4:
                    kq_proj(1, [c], "q")      # lazy Q-p1 for this chunk
                csl = slice(c * 512, (c + 1) * 512)
                ex_tiles = []
                for jt in range(JT):
                    jsl = slice(jt * 128, (jt + 1) * 128)
                    sc = ps.tile([128, 2, 512], F32, tag="sc",
                                 name=f"sc{p}{c}{jt}")
                    nc.tensor.matmul(
                        sc[:, 0, :], kt_sb[p][0:64, jsl], qt_sb[p][0:64, csl],
                        start=True, stop=True, tile_position=(0, 0))
                    nc.tensor.matmul(
                        sc[:, 1, :], kt_sb[p][64:128, jsl],
                        qt_sb[p][64:128, csl],
                        start=True, stop=True, tile_position=(64, 0))
                    ex = expp.tile([128, 2, 512], BF16, tag="ex",
                                   name=f"ex{p}{c}{jt}")
                    nc.scalar.activation(
                        out=ex, in_=sc,
                        func=mybir.ActivationFunctionType.Exp, scale=SCALE)
                    ex_tiles.append(ex)
                    # spread late phase-A work through the early windows
                    if i == 0 and jt % 4 == 3 and pend_kp1:
                        kq_proj(1, [pend_kp1.pop(0)], "k")
                        if pend_q0:
                            kq_proj(0, [pend_q0.pop(0)], "q")
                    elif i == 1:
                        v_proj([jt])
                if pending is not None:
                    tail(*pending)
                pending = (p, c, ex_tiles)
            tail(*pending)

    nc.compile()
    return nc


_NC = None


def _get_nc():
    global _NC
    if _NC is None:
        _NC = _build_nc()
    return _NC


def make_in_maps(query, key, value, Wq, Wk, Wv, Wo):
    bf = ml_dtypes.bfloat16
    xqT = [np.ascontiguousarray(np.asarray(query[b], dtype=np.float32).T.astype(bf))
           for b in range(B)]
    xkT = [np.ascontiguousarray(np.asarray(key[b], dtype=np.float32).T.astype(bf))
           for b in range(B)]
    xvT = [np.ascontiguousarray(np.asarray(value[b], dtype=np.float32).T.astype(bf))
           for b in range(B)]
    Wq = np.asarray(Wq, dtype=np.float32)
    Wk = np.asarray(Wk, dtype=np.float32)
    Wv = np.asarray(Wv, dtype=np.float32)
    Wo = np.asarray(Wo, dtype=np.float32)

    in_maps = []
    for core in range(8):
        b, g = divmod(core, 4)
        sl = slice(g * DG, (g + 1) * DG)
        in_maps.append({
            "xq": xqT[b],
            "xk": xkT[b],
            "xv": xvT[b],
            "wq": np.ascontiguousarray(Wq[:, sl].astype(bf)),
            "wk": np.ascontiguousarray(Wk[:, sl].astype(bf)),
            "wv": np.ascontiguousarray(Wv[:, sl].astype(bf)),
            "wo": np.ascontiguousarray(Wo[sl, :].astype(bf)),
        })
    return in_maps


def combine_results(results):
    out = np.zeros((B, S, D), dtype=np.float32)
    for core in range(8):
        out[core // 4] += results[core]["out"].astype(np.float32)
    return out


def kernel(query, key, value, Wq, Wk, Wv, Wo, _trace=False):
    from concourse import bass_utils

    nc = _get_nc()
    in_maps = make_in_maps(query, key, value, Wq, Wk, Wv, Wo)
    r = bass_utils.run_bass_kernel_spmd(
        nc, in_maps, core_ids=list(range(8)), trace=_trace
    )
    kernel.last_results = r
    return combine_results(r.results)


# revision 11
# speedup vs baseline: 1.0220x; 1.0220x over previous
"""Trainium2 Bass kernel for 16-head MultiHeadAttention (B=2, S=2048, D=1024, f32).

Sharding: 8 cores = 2 (batch) x 4 (head groups of 4 heads).
Each core gets col-shards of Wq/Wk/Wv + a row-shard of Wo, computes a full
[2048,1024] partial output; the host sums the 8 partials (4 per batch) into
[2,2048,1024]. All device data bf16 (f32 PSUM accumulation), output bf16.

The ACT engine (exp over 4 x 2048 x 2048 scores: 128 instrs x ~1.15us =
147us) paces the kernel, and warm-PE work (~143us) just fits under it - but
ONLY if the PE stays dense enough that the HAM clock gate keeps it at
2.4GHz. So everything is interleaved at jt (128-key-tile) granularity:

  per chunk i (= head-pair p, 512-query chunk c), per jt:
      scores(i,jt)   - 2 heads row-packed via tile_position, "sc" ring (2x2
                       PSUM banks)
      exp(i,jt)      - ACT, bf16 out
      insert(i,jt)   - one projection accumulator (K/Q/V head-pair work) or
                       one Wo accumulator, scheduled so its DMA has landed
      AV(i-1, jt)    - previous chunk's attention*V: V_aug ([j,65], ones
                       col -> softmax denominators) stationary, expT moving,
                       one [65,512] accumulator per head ("av" ring, 2x1
                       banks, pair held for the whole chunk)
  after the jt loop: normalize chunk i-1 (reciprocal row + gpsimd partition
  broadcast + multiply into outT bf16).

Wo (K=256 accumulation over both head pairs -> one bf16 output) rides the
"acc" ring: wo(c4) interleaved in chunk 6, wo(c5) in chunk 7, wo(c6)+wo(c7)
plus AV(c7) in the tail.
"""

import sys

import numpy as np

if "/opt/trn_rl_repo" not in sys.path:
    sys.path.insert(0, "/opt/trn_rl_repo")

import ml_dtypes

import concourse.bacc as bacc
import concourse.mybir as mybir
import concourse.tile as tile

F32 = mybir.dt.float32
BF16 = mybir.dt.bfloat16

B, S, D, H = 2, 2048, 1024, 16
DK = D // H          # 64
HL = 4               # heads per core
DG = HL * DK         # 256
SCALE = 0.125        # 1/sqrt(DK)

ET = D // 128        # 8 e-tiles (contraction over D)
JT = S // 128        # 16 j-tiles (keys)
QC = S // 512        # 4 q-chunks


def _build_nc():
    nc = bacc.Bacc("TRN2", target_bir_lowering=False, debug=False)

    xq = nc.dram_tensor("xq", [D, S], BF16, kind="ExternalInput").ap()
    xk = nc.dram_tensor("xk", [D, S], BF16, kind="ExternalInput").ap()
    xv = nc.dram_tensor("xv", [D, S], BF16, kind="ExternalInput").ap()
    wq = nc.dram_tensor("wq", [D, DG], BF16, kind="ExternalInput").ap()
    wk = nc.dram_tensor("wk", [D, DG], BF16, kind="ExternalInput").ap()
    wv = nc.dram_tensor("wv", [D, DG], BF16, kind="ExternalInput").ap()
    wo = nc.dram_tensor("wo", [DG, D], BF16, kind="ExternalInput").ap()
    out = nc.dram_tensor("out", [S, D], BF16, kind="ExternalOutput").ap()

    with tile.TileContext(nc) as tc:
        with (
            tc.tile_pool(name="wpool", bufs=1) as wpool,
            tc.tile_pool(name="xin", bufs=1) as xin,
            tc.tile_pool(name="proj", bufs=1) as proj,
            tc.tile_pool(name="expp", bufs=20) as expp,
            tc.tile_pool(name="nrm", bufs=4) as nrm,
            tc.tile_pool(name="osbp", bufs=2) as osbp,
            tc.tile_pool(name="ps", bufs=1, space="PSUM") as ps,
        ):
            # ---- ACT warmup: force the Exp table load at t=0 --------------
            wu_in = wpool.tile([128, 16], F32, tag="wu", name="wu_in")
            nc.vector.memset(wu_in, 0.0)
            wu_out = wpool.tile([128, 16], BF16, tag="wuo", name="wu_out")
            nc.scalar.activation(
                out=wu_out, in_=wu_in,
                func=mybir.ActivationFunctionType.Exp, scale=1.0,
            )

            # ---- weight tiles ---------------------------------------------
            wk_sb = [wpool.tile([128, DG], BF16, tag=f"wk{e}", name=f"wk{e}")
                     for e in range(ET)]
            wq_sb = [wpool.tile([128, DG], BF16, tag=f"wq{e}", name=f"wq{e}")
                     for e in range(ET)]
            wv_sb = [wpool.tile([128, DG], BF16, tag=f"wv{e}", name=f"wv{e}")
                     for e in range(ET)]
            wo_sb = [wpool.tile([128, D], BF16, tag=f"wo{p}", name=f"wo{p}")
                     for p in range(2)]

            # ---- persistent activation tiles ------------------------------
            kt_sb = [proj.tile([128, S], BF16, tag=f"kt{p}", name=f"kt{p}")
                     for p in range(2)]
            qt_sb = [proj.tile([128, S], BF16, tag=f"qt{p}", name=f"qt{p}")
                     for p in range(2)]
            v_sb = proj.tile([128, JT, HL, DK + 1], BF16, tag="v", name="v_sb")
            nc.vector.memset(v_sb[:, :, :, DK:DK + 1], 1.0)
            outt_sb = [proj.tile([128, S], BF16, tag=f"ot{p}", name=f"outt{p}")
                       for p in range(2)]

            # ---- DMA emission (sync+gpsimd round-robin) -------------------
            queues = [nc.sync, nc.gpsimd]
            rr = [0]

            def dq(dst, src):
                queues[rr[0] % 2].dma_start(dst, src)
                rr[0] += 1

            xk_t = [xin.tile([128, S], BF16, tag=f"xk{e}", name=f"xk{e}")
                    for e in range(ET)]
            xq_t = [xin.tile([128, S], BF16, tag=f"xq{e}", name=f"xq{e}")
                    for e in range(ET)]
            xv_t = [xin.tile([128, S], BF16, tag=f"xv{e}", name=f"xv{e}")
                    for e in range(ET)]

            def dx(ts, dram, c):
                sl = slice(c * 512, (c + 1) * 512)
                for e in range(ET):
                    dq(ts[e][:, sl], dram[e * 128:(e + 1) * 128, sl])

            def dw(ts, dram):
                for e in range(ET):
                    dq(ts[e], dram[e * 128:(e + 1) * 128, :])

            dw(wk_sb, wk)
            dx(xk_t, xk, 0)
            dw(wq_sb, wq)
            dx(xq_t, xq, 0)
            dx(xk_t, xk, 1)
            dx(xk_t, xk, 2)
            dx(xk_t, xk, 3)
            dx(xq_t, xq, 1)
            dw(wv_sb, wv)
            dx(xv_t, xv, 0)
            dx(xv_t, xv, 1)
            dx(xv_t, xv, 2)
            dx(xv_t, xv, 3)
            dx(xq_t, xq, 2)
            dx(xq_t, xq, 3)
            for p in range(2):
                dq(wo_sb[p], wo[p * 128:(p + 1) * 128, :])

            # ---- emission helpers -----------------------------------------
            def kq_acc(nm, p, c):
                w_sb, x_t, dst = {
                    "k": (wk_sb, xk_t, kt_sb), "q": (wq_sb, xq_t, qt_sb),
                }[nm]
                csl = slice(c * 512, (c + 1) * 512)
                acc = ps.tile([128, 512], F32, tag="acc", name=f"a{nm}{p}{c}")
                for e in range(ET):
                    nc.tensor.matmul(
                        acc, w_sb[e][:, p * 128:(p + 1) * 128], x_t[e][:, csl],
                        start=(e == 0), stop=(e == ET - 1))
                nc.vector.tensor_copy(dst[p][:, csl], acc)

            def v_acc(jt):
                jsl = slice(jt * 128, (jt + 1) * 128)
                accv = ps.tile([128, DG], F32, tag="acc", name=f"pv{jt}")
                for e in range(ET):
                    nc.tensor.matmul(
                        accv, xv_t[e][:, jsl], wv_sb[e],
                        start=(e == 0), stop=(e == ET - 1))
                nc.vector.tensor_copy(
                    v_sb[:, jt, :, 0:DK],
                    accv.rearrange("j (h d) -> j h d", h=HL))

            osb_state = {}

            def wo_acc(c, k8):
                """One Wo accumulator: query tile c*4+k8//2, d-half k8%2."""
                qg, ch = c * 4 + k8 // 2, k8 % 2
                chsl = slice(ch * 512, (ch + 1) * 512)
                if ch == 0:
                    osb_state[(c, qg)] = osbp.tile(
                        [128, D], BF16, tag="osb", name=f"osb{c}{qg}")
                osb = osb_state[(c, qg)]
                acc = ps.tile([128, 512], F32, tag="acc", name=f"po{c}{k8}")
                nc.tensor.matmul(
                    acc, outt_sb[0][:, qg * 128:(qg + 1) * 128],
                    wo_sb[0][:, chsl], start=True, stop=False)
                nc.tensor.matmul(
                    acc, outt_sb[1][:, qg * 128:(qg + 1) * 128],
                    wo_sb[1][:, chsl], start=False, stop=True)
                nc.vector.tensor_copy(osb[:, chsl], acc)
                if ch == 1:
                    nc.sync.dma_start(out[qg * 128:(qg + 1) * 128, :], osb)

            def norm(p, c, avA, avB):
                csl = slice(c * 512, (c + 1) * 512)
                for h, av in ((0, avA), (1, avB)):
                    rrow = nrm.tile([1, 512], F32, tag="rr", name=f"rr{p}{c}{h}")
                    nc.vector.reciprocal(rrow, av[DK:DK + 1, :])
                    rbc = nrm.tile([64, 512], F32, tag="rbc", name=f"rb{p}{c}{h}")
                    nc.gpsimd.partition_broadcast(rbc, rrow)
                    nc.vector.tensor_tensor(
                        outt_sb[p][h * 64:(h + 1) * 64, csl],
                        av[0:DK, :], rbc, mybir.AluOpType.mult)

            # insert schedule: at most one ~1.7us accumulator per jt window,
            # placed after its input DMA lands.
            inserts = {
                0: {1: lambda: kq_acc("k", 0, 1), 3: lambda: kq_acc("k", 0, 2),
                    5: lambda: kq_acc("k", 0, 3), 7: lambda: kq_acc("q", 0, 1),
                    9: lambda: kq_acc("k", 1, 0), 11: lambda: kq_acc("k", 1, 1),
                    13: lambda: kq_acc("k", 1, 2), 15: lambda: kq_acc("k", 1, 3)},
                1: {**{jt: (lambda jt=jt: (v_acc(0), v_acc(1)) if jt == 0
                            else v_acc(jt + 1)) for jt in range(15)},
                    15: lambda: kq_acc("q", 0, 2)},
                2: {1: lambda: kq_acc("q", 0, 3)},
                3: {1: lambda: kq_acc("q", 1, 0), 3: lambda: kq_acc("q", 1, 1),
                    5: lambda: kq_acc("q", 1, 2), 7: lambda: kq_acc("q", 1, 3)},
                6: {2 * k + 1: (lambda k=k: wo_acc(0, k)) for k in range(8)},
                7: {2 * k + 1: (lambda k=k: wo_acc(1, k)) for k in range(8)},
            }

            # ---- main flow ------------------------------------------------
            kq_acc("k", 0, 0)
            kq_acc("q", 0, 0)

            prev = None        # (p, c, ex_tiles) of chunk i-1
            prev_av = None     # its (avA, avB) accumulators
            for i in range(8):
                p, c = i // 4, i % 4
                csl = slice(c * 512, (c + 1) * 512)
                if prev is not None:
                    pp, pc, _ = prev
                    avA = ps.tile([128, 512], F32, tag="av", name=f"avA{pp}{pc}")
                    avB = ps.tile([128, 512], F32, tag="av", name=f"avB{pp}{pc}")
                    prev_av = (avA, avB)
                ex_tiles = []
                for jt in range(JT):
                    jsl = slice(jt * 128, (jt + 1) * 128)
                    sc = ps.tile([128, 2, 512], F32, tag="sc",
                                 name=f"sc{p}{c}{jt}")
                    nc.tensor.matmul(
                        sc[:, 0, :], kt_sb[p][0:64, jsl], qt_sb[p][0:64, csl],
                        start=True, stop=True, tile_position=(0, 0))
                    nc.tensor.matmul(
                        sc[:, 1, :], kt_sb[p][64:128, jsl],
                        qt_sb[p][64:128, csl],
                        start=True, stop=True, tile_position=(64, 0))
                    ex = expp.tile([128, 2, 512], BF16, tag="ex",
                                   name=f"ex{p}{c}{jt}")
                    nc.scalar.activation(
                        out=ex, in_=sc,
                        func=mybir.ActivationFunctionType.Exp, scale=SCALE)
                    ex_tiles.append(ex)
                    fn = inserts.get(i, {}).get(jt)
                    if fn is not None:
                        fn()
                    if prev is not None:
                        pp, pc, pex = prev
                        for h, av in ((0, prev_av[0]), (1, prev_av[1])):
                            nc.tensor.matmul(
                                av[0:DK + 1, :], v_sb[:, jt, 2 * pp + h, :],
                                pex[jt][:, h, :],
                                start=(jt == 0), stop=(jt == JT - 1))
                if prev is not None:
                    norm(prev[0], prev[1], *prev_av)
                prev = (p, c, ex_tiles)

            # ---- tail: AV + norm for the last chunk, then wo(c6), wo(c7) --
            avA = ps.tile([128, 512], F32, tag="av", name="avA13")
            avB = ps.tile([128, 512], F32, tag="av", name="avB13")
            for jt in range(JT):
                for h, av in ((0, avA), (1, avB)):
                    nc.tensor.matmul(
                        av[0:DK + 1, :], v_sb[:, jt, 2 + h, :],
                        prev[2][jt][:, h, :],
                        start=(jt == 0), stop=(jt == JT - 1))
            norm(1, 3, avA, avB)
            for k in range(8):
                wo_acc(2, k)
            for k in range(8):
                wo_acc(3, k)

    nc.compile()
    return nc


_NC = None


def _get_nc():
    global _NC
    if _NC is None:
        _NC = _build_nc()
    return _NC


def make_in_maps(query, key, value, Wq, Wk, Wv, Wo):
    bf = ml_dtypes.bfloat16
    xqT = [np.ascontiguousarray(np.asarray(query[b], dtype=np.float32).T.astype(bf))
           for b in range(B)]
    xkT = [np.ascontiguousarray(np.asarray(key[b], dtype=np.float32).T.astype(bf))
           for b in range(B)]
    xvT = [np.ascontiguousarray(np.asarray(value[b], dtype=np.float32).T.astype(bf))
           for b in range(B)]
    Wq = np.asarray(Wq, dtype=np.float32)
    Wk = np.asarray(Wk, dtype=np.float32)
    Wv = np.asarray(Wv, dtype=np.float32)
    Wo = np.asarray(Wo, dtype=np.float32)

    in_maps = []
    for core in range(8):
        b, g = divmod(core, 4)
        sl = slice(g * DG, (g + 1) * DG)
        in_maps.append({
            "xq": xqT[b],
            "xk": xkT[b],
            "xv": xvT[b],
            "wq": np.ascontiguousarray(Wq[:, sl].astype(bf)),
            "wk": np.ascontiguousarray(Wk[:, sl].astype(bf)),
            "wv": np.ascontiguousarray(Wv[:, sl].astype(bf)),
            "wo": np.ascontiguousarray(Wo[sl, :].astype(bf)),
        })
    return in_maps


def combine_results(results):
    out = np.zeros((B, S, D), dtype=np.float32)
    for core in range(8):
        out[core // 4] += results[core]["out"].astype(np.float32)
    return out


def kernel(query, key, value, Wq, Wk, Wv, Wo, _trace=False):
    from concourse import bass_utils

    nc = _get_nc()
    in_maps = make_in_maps(query, key, value, Wq, Wk, Wv, Wo)
    r = bass_utils.run_bass_kernel_spmd(
        nc, in_maps, core_ids=list(range(8)), trace=_trace
    )
    kernel.last_results = r
    return combine_results(r.results)


# revision 15
# speedup vs baseline: 1.4225x; 1.3919x over previous
"""Trainium2 Bass kernel for 16-head MultiHeadAttention (B=2, S=2048, D=1024, f32).

Sharding: 8 cores = 2 (batch) x 4 (head groups of 4 heads).
Each core gets col-shards of Wq/Wk/Wv ([1024,256]) + a row-shard of Wo
([256,1024]), computes a full [2048,1024] partial output; the host sums the
8 partials (4 per batch element) into [2,2048,1024].

All device data is bf16 (f32 accumulation in PSUM); the host converts inputs
and upconverts the bf16 partial outputs. rel-err budget 2e-2 >> bf16 noise.

On-device pipeline (per core):
  KT/QT = W^T @ x^T                    -> [128(=2 heads x 64), 2048] bf16
  V     = xv^T-tiles @ Wv directly     -> v_sb[j, jt, head, 65] (65th col = 1
          so the AV matmul emits softmax denominators for free)
  sT    = KT_h^T-slice @ QT_h-slice, two heads row-packed in the PE array
          via tile_position (0,0)/(64,0)          [128 j, 2, 512 q] PSUM f32
  expT  = exp(0.125 * sT) on ACT -> bf16 SBUF (ACT is the pacing engine:
          128 instrs x ~1.15us; everything else hides behind it)
  AV    : transposed orientation: lhsT = expT[j, q-slice], rhs = V_aug[j, 65]
          -> out[q, 65] PSUM, full 128x128 PE occupancy (2x fewer cycles than
          the dk-partition orientation). Column 64 = softmax denominator.
  norm  : per-partition reciprocal + tensor_scalar multiply (DVE)
  outT  : PE-transpose of normalized out back to [dg, q] for the Wo matmul
  out  += outT_p^T @ Wo_p accumulated over BOTH head pairs (K=256) -> one
          [2048, 1024] bf16 partial per core (half the output traffic).
"""

import sys

import numpy as np

if "/opt/trn_rl_repo" not in sys.path:
    sys.path.insert(0, "/opt/trn_rl_repo")

import ml_dtypes

import concourse.bacc as bacc
import concourse.mybir as mybir
import concourse.tile as tile
from concourse.masks import make_identity

F32 = mybir.dt.float32
BF16 = mybir.dt.bfloat16

B, S, D, H = 2, 2048, 1024, 16
DK = D // H          # 64
HL = 4               # heads per core
DG = HL * DK         # 256
SCALE = 0.125        # 1/sqrt(DK)

ET = D // 128        # 8 e-tiles (contraction over D)
JT = S // 128        # 16 j-tiles (keys)
QC = S // 512        # 4 q-chunks


def _build_nc():
    nc = bacc.Bacc("TRN2", target_bir_lowering=False, debug=False)

    xq = nc.dram_tensor("xq", [D, S], BF16, kind="ExternalInput").ap()
    xk = nc.dram_tensor("xk", [D, S], BF16, kind="ExternalInput").ap()
    xv = nc.dram_tensor("xv", [D, S], BF16, kind="ExternalInput").ap()
    wq = nc.dram_tensor("wq", [D, DG], BF16, kind="ExternalInput").ap()
    wk = nc.dram_tensor("wk", [D, DG], BF16, kind="ExternalInput").ap()
    wv = nc.dram_tensor("wv", [D, DG], BF16, kind="ExternalInput").ap()
    wo = nc.dram_tensor("wo", [DG, D], BF16, kind="ExternalInput").ap()
    out = nc.dram_tensor("out", [S, D], BF16, kind="ExternalOutput").ap()

    with tile.TileContext(nc) as tc:
        with (
            tc.tile_pool(name="wpool", bufs=1) as wpool,
            tc.tile_pool(name="xin", bufs=1) as xin,
            tc.tile_pool(name="proj", bufs=1) as proj,
            tc.tile_pool(name="expp", bufs=22) as expp,
            tc.tile_pool(name="nrm", bufs=6) as nrm,
            tc.tile_pool(name="osbp", bufs=2) as osbp,
            # PSUM: acc ring (AV groups / transposes / Wo) on banks 0-2;
            # opened before ps_a so phase A's late work (banks 3-7) never
            # blocks the attention accumulators.
            tc.tile_pool(name="ps_acc", bufs=3, space="PSUM") as ps_acc,
        ):
            # ---- ACT warmup: force the Exp table load at t=0 --------------
            wu_in = wpool.tile([128, 16], F32, tag="wu", name="wu_in")
            nc.vector.memset(wu_in, 0.0)
            wu_out = wpool.tile([128, 16], BF16, tag="wuo", name="wu_out")
            nc.scalar.activation(
                out=wu_out, in_=wu_in,
                func=mybir.ActivationFunctionType.Exp, scale=1.0,
            )

            # ---- constants + weight tiles ---------------------------------
            wk_sb = [wpool.tile([128, DG], BF16, tag=f"wk{e}", name=f"wk{e}")
                     for e in range(ET)]
            wq_sb = [wpool.tile([128, DG], BF16, tag=f"wq{e}", name=f"wq{e}")
                     for e in range(ET)]
            wv_sb = [wpool.tile([128, DG], BF16, tag=f"wv{e}", name=f"wv{e}")
                     for e in range(ET)]
            wo_sb = [wpool.tile([128, D], BF16, tag=f"wo{p}", name=f"wo{p}")
                     for p in range(2)]

            ident_f = wpool.tile([128, 128], F32, tag="ident_f", name="ident_f")
            make_identity(nc, ident_f)
            ident = wpool.tile([128, 128], BF16, tag="ident", name="ident")
            nc.vector.tensor_copy(ident, ident_f)

            # ---- persistent activation tiles ------------------------------
            kt_sb = [proj.tile([128, S], BF16, tag=f"kt{p}", name=f"kt{p}")
                     for p in range(2)]
            qt_sb = [proj.tile([128, S], BF16, tag=f"qt{p}", name=f"qt{p}")
                     for p in range(2)]
            v_sb = proj.tile([128, JT, HL, DK + 1], BF16, tag="v", name="v_sb")
            nc.vector.memset(v_sb[:, :, :, DK:DK + 1], 1.0)
            outt_sb = [proj.tile([128, S], BF16, tag=f"ot{p}", name=f"outt{p}")
                       for p in range(2)]

            # ---- DMA emission (3 queues round-robin) ----------------------
            # Order tuned so exp(c0) can start ~9.5us in: wk, xk[c0], wq,
            # xq[c0], xk[c1..c3] (keeps exp c0 fed), xv[c0], wv, then the
            # rest column-interleaved, wo last.
            queues = [nc.sync, nc.gpsimd]
            rr = [0]

            def dq(dst, src):
                queues[rr[0] % 2].dma_start(dst, src)
                rr[0] += 1

            xk_t = [xin.tile([128, S], BF16, tag=f"xk{e}", name=f"xk{e}")
                    for e in range(ET)]
            xq_t = [xin.tile([128, S], BF16, tag=f"xq{e}", name=f"xq{e}")
                    for e in range(ET)]
            xv_t = [xin.tile([128, S], BF16, tag=f"xv{e}", name=f"xv{e}")
                    for e in range(ET)]

            def dx(ts, dram, c):
                sl = slice(c * 512, (c + 1) * 512)
                for e in range(ET):
                    dq(ts[e][:, sl], dram[e * 128:(e + 1) * 128, sl])

            for e in range(ET):
                dq(wk_sb[e], wk[e * 128:(e + 1) * 128, :])
            dx(xk_t, xk, 0)
            for e in range(ET):
                dq(wq_sb[e], wq[e * 128:(e + 1) * 128, :])
            dx(xq_t, xq, 0)
            dx(xk_t, xk, 1)
            dx(xk_t, xk, 2)
            dx(xk_t, xk, 3)
            dx(xv_t, xv, 0)
            for e in range(ET):
                dq(wv_sb[e], wv[e * 128:(e + 1) * 128, :])
            dx(xq_t, xq, 1)
            dx(xv_t, xv, 1)
            dx(xq_t, xq, 2)
            dx(xv_t, xv, 2)
            dx(xq_t, xq, 3)
            dx(xv_t, xv, 3)
            for p in range(2):
                dq(wo_sb[p], wo[p * 128:(p + 1) * 128, :])

            # ---- phase A: projections -------------------------------------
            with tc.tile_pool(name="ps_a", bufs=1, space="PSUM") as ps_a:
                # K/Q head-pair 0 first (its PSUM banks are reused by the
                # score tiles, so they must drain early).
                for c in range(QC):
                    csl = slice(c * 512, (c + 1) * 512)
                    for w_sb, x_t, dst, nm in (
                        (wk_sb, xk_t, kt_sb, "k"),
                        (wq_sb, xq_t, qt_sb, "q"),
                    ):
                        acc = ps_a.tile([128, 512], F32, tag="paq", bufs=4,
                                        name=f"a{nm}0{c}")
                        for e in range(ET):
                            nc.tensor.matmul(
                                acc, w_sb[e][:, 0:128], x_t[e][:, csl],
                                start=(e == 0), stop=(e == ET - 1),
                            )
                        nc.vector.tensor_copy(dst[0][:, csl], acc)
                # V directly in [seq, head*dk] layout (feeds AV's rhs).
                for jt in range(JT):
                    jsl = slice(jt * 128, (jt + 1) * 128)
                    accv = ps_a.tile([128, DG], F32, tag="pal", bufs=1,
                                     name=f"av_{jt}")
                    for e in range(ET):
                        nc.tensor.matmul(
                            accv, xv_t[e][:, jsl], wv_sb[e],
                            start=(e == 0), stop=(e == ET - 1),
                        )
                    nc.vector.tensor_copy(
                        v_sb[:, jt, :, 0:DK],
                        accv.rearrange("j (h d) -> j h d", h=HL),
                    )
                # K/Q head-pair 1 (needed only from the second half).
                for c in range(QC):
                    csl = slice(c * 512, (c + 1) * 512)
                    for w_sb, x_t, dst, nm in (
                        (wk_sb, xk_t, kt_sb, "k"),
                        (wq_sb, xq_t, qt_sb, "q"),
                    ):
                        acc = ps_a.tile([128, 512], F32, tag="pal", bufs=1,
                                        name=f"a{nm}1{c}")
                        for e in range(ET):
                            nc.tensor.matmul(
                                acc, w_sb[e][:, 128:256], x_t[e][:, csl],
                                start=(e == 0), stop=(e == ET - 1),
                            )
                        nc.vector.tensor_copy(dst[1][:, csl], acc)

            # ---- phase B: attention + output projection -------------------
            def tail(p, c, ex_tiles):
                """AV waves + normalize + transpose (+ Wo when p==1)."""
                hA, hB = 2 * p, 2 * p + 1
                for k in range(4):          # one 128-query tile per wave
                    qsl = slice(k * 128, (k + 1) * 128)
                    accA = ps_acc.tile([128, DK + 1], F32, tag="acc",
                                       name=f"avA{p}{c}{k}")
                    accB = ps_acc.tile([128, DK + 1], F32, tag="acc",
                                       name=f"avB{p}{c}{k}")
                    for jt in range(JT):
                        nc.tensor.matmul(
                            accA, ex_tiles[jt][:, 0, qsl], v_sb[:, jt, hA, :],
                            start=(jt == 0), stop=(jt == JT - 1),
                        )
                        nc.tensor.matmul(
                            accB, ex_tiles[jt][:, 1, qsl], v_sb[:, jt, hB, :],
                            start=(jt == 0), stop=(jt == JT - 1),
                        )
                    recA = nrm.tile([128, 1], F32, tag="rec", name=f"rA{p}{c}{k}")
                    recB = nrm.tile([128, 1], F32, tag="rec", name=f"rB{p}{c}{k}")
                    nc.vector.reciprocal(recA, accA[:, DK:DK + 1])
                    nc.vector.reciprocal(recB, accB[:, DK:DK + 1])
                    nt = nrm.tile([128, 2, DK], BF16, tag="nt", name=f"nt{p}{c}{k}")
                    nc.vector.tensor_scalar(
                        nt[:, 0, :], accA[:, 0:DK], recA, None,
                        mybir.AluOpType.mult)
                    nc.vector.tensor_scalar(
                        nt[:, 1, :], accB[:, 0:DK], recB, None,
                        mybir.AluOpType.mult)
                    pt = ps_acc.tile([128, 128], BF16, tag="acc",
                                     name=f"pt{p}{c}{k}")
                    nc.tensor.transpose(pt, nt.rearrange("q h d -> q (h d)"),
                                        ident)
                    qg = c * 4 + k
                    nc.vector.tensor_copy(
                        outt_sb[p][:, qg * 128:(qg + 1) * 128], pt)
                if p == 1:
                    for k in range(4):
                        qg = c * 4 + k
                        osb = osbp.tile([128, D], BF16, tag="osb",
                                        name=f"osb{c}{k}")
                        for ch in range(2):
                            chsl = slice(ch * 512, (ch + 1) * 512)
                            acc = ps_acc.tile([128, 512], F32, tag="acc",
                                              name=f"po{c}{k}{ch}")
                            nc.tensor.matmul(
                                acc, outt_sb[0][:, qg * 128:(qg + 1) * 128],
                                wo_sb[0][:, chsl], start=True, stop=False)
                            nc.tensor.matmul(
                                acc, outt_sb[1][:, qg * 128:(qg + 1) * 128],
                                wo_sb[1][:, chsl], start=False, stop=True)
                            nc.vector.tensor_copy(osb[:, chsl], acc)
                        nc.sync.dma_start(
                            out[qg * 128:(qg + 1) * 128, :], osb)

            with tc.tile_pool(name="ps_sc", bufs=2, space="PSUM") as ps_sc:
                pending = None
                for p in range(2):
                    for c in range(QC):
                        csl = slice(c * 512, (c + 1) * 512)
                        ex_tiles = []
                        for jt in range(JT):
                            jsl = slice(jt * 128, (jt + 1) * 128)
                            sc = ps_sc.tile([128, 2, 512], F32, tag="sc",
                                            name=f"sc{p}{c}{jt}")
                            nc.tensor.matmul(
                                sc[:, 0, :], kt_sb[p][0:64, jsl],
                                qt_sb[p][0:64, csl],
                                start=True, stop=True, tile_position=(0, 0))
                            nc.tensor.matmul(
                                sc[:, 1, :], kt_sb[p][64:128, jsl],
                                qt_sb[p][64:128, csl],
                                start=True, stop=True, tile_position=(64, 0))
                            ex = expp.tile([128, 2, 512], BF16, tag="ex",
                                           name=f"ex{p}{c}{jt}")
                            nc.scalar.activation(
                                out=ex, in_=sc,
                                func=mybir.ActivationFunctionType.Exp,
                                scale=SCALE)
                            ex_tiles.append(ex)
                        if pending is not None:
                            tail(*pending)
                        pending = (p, c, ex_tiles)
                tail(*pending)

    nc.compile()
    return nc


_NC = None


def _get_nc():
    global _NC
    if _NC is None:
        _NC = _build_nc()
    return _NC


def make_in_maps(query, key, value, Wq, Wk, Wv, Wo):
    bf = ml_dtypes.bfloat16
    xqT = [np.ascontiguousarray(np.asarray(query[b], dtype=np.float32).T.astype(bf))
           for b in range(B)]
    xkT = [np.ascontiguousarray(np.asarray(key[b], dtype=np.float32).T.astype(bf))
           for b in range(B)]
    xvT = [np.ascontiguousarray(np.asarray(value[b], dtype=np.float32).T.astype(bf))
           for b in range(B)]
    Wq = np.asarray(Wq, dtype=np.float32)
    Wk = np.asarray(Wk, dtype=np.float32)
    Wv = np.asarray(Wv, dtype=np.float32)
    Wo = np.asarray(Wo, dtype=np.float32)

    in_maps = []
    for core in range(8):
        b, g = divmod(core, 4)
        sl = slice(g * DG, (g + 1) * DG)
        in_maps.append({
            "xq": xqT[b],
            "xk": xkT[b],
            "xv": xvT[b],
            "wq": np.ascontiguousarray(Wq[:, sl].astype(bf)),
            "wk": np.ascontiguousarray(Wk[:, sl].astype(bf)),
            "wv": np.ascontiguousarray(Wv[:, sl].astype(bf)),
            "wo": np.ascontiguousarray(Wo[sl, :].astype(bf)),
        })
    return in_maps


def combine_results(results):
    out = np.zeros((B, S, D), dtype=np.float32)
    for core in range(8):
        out[core // 4] += results[core]["out"].astype(np.float32)
    return out


def kernel(query, key, value, Wq, Wk, Wv, Wo, _trace=False):
    from concourse import bass_utils

    nc = _get_nc()
    in_maps = make_in_maps(query, key, value, Wq, Wk, Wv, Wo)
    r = bass_utils.run_bass_kernel_spmd(
        nc, in_maps, core_ids=list(range(8)), trace=_trace
    )
    kernel.last_results = r
    return combine_results(r.results)


# revision 20
# speedup vs baseline: 1.4534x; 1.0217x over previous
"""Trainium2 Bass kernel for 16-head MultiHeadAttention (B=2, S=2048, D=1024, f32).

Sharding: 8 cores = 2 (batch) x 4 (head groups of 4 heads).
Each core gets col-shards of Wq/Wk/Wv ([1024,256]) + a row-shard of Wo
([256,1024]), computes a full [2048,1024] partial output; the host sums the
8 partials (4 per batch element) into [2,2048,1024].

All device data is bf16 (f32 accumulation in PSUM); the host converts inputs
and upconverts the bf16 partial outputs. rel-err budget 2e-2 >> bf16 noise.

On-device pipeline (per core):
  KT/QT = W^T @ x^T                    -> [128(=2 heads x 64), 2048] bf16
  V     = xv^T-tiles @ Wv directly     -> v_sb[j, jt, head, 65] (65th col = 1
          so the AV matmul emits softmax denominators for free)
  sT    = KT_h^T-slice @ QT_h-slice, two heads row-packed in the PE array
          via tile_position (0,0)/(64,0)          [128 j, 2, 512 q] PSUM f32
  expT  = exp(0.125 * sT) on ACT -> bf16 SBUF (ACT is the pacing engine:
          128 instrs x ~1.15us; everything else hides behind it)
  AV    : transposed orientation: lhsT = expT[j, q-slice], rhs = V_aug[j, 65]
          -> out[q, 65] PSUM, full 128x128 PE occupancy (2x fewer cycles than
          the dk-partition orientation). Column 64 = softmax denominator.
  norm  : per-partition reciprocal + tensor_scalar multiply (DVE)
  outT  : PE-transpose of normalized out back to [dg, q] for the Wo matmul
  out  += outT_p^T @ Wo_p accumulated over BOTH head pairs (K=256) -> one
          [2048, 1024] bf16 partial per core (half the output traffic).
"""

import sys

import numpy as np

if "/opt/trn_rl_repo" not in sys.path:
    sys.path.insert(0, "/opt/trn_rl_repo")

import ml_dtypes

import concourse.bacc as bacc
import concourse.mybir as mybir
import concourse.tile as tile
from concourse.masks import make_identity

F32 = mybir.dt.float32
BF16 = mybir.dt.bfloat16

B, S, D, H = 2, 2048, 1024, 16
DK = D // H          # 64
HL = 4               # heads per core
DG = HL * DK         # 256
SCALE = 0.125        # 1/sqrt(DK)

ET = D // 128        # 8 e-tiles (contraction over D)
JT = S // 128        # 16 j-tiles (keys)
QC = S // 512        # 4 q-chunks


def _build_nc():
    nc = bacc.Bacc("TRN2", target_bir_lowering=False, debug=False)

    xq = nc.dram_tensor("xq", [D, S], BF16, kind="ExternalInput").ap()
    xk = nc.dram_tensor("xk", [D, S], BF16, kind="ExternalInput").ap()
    xv = nc.dram_tensor("xv", [D, S], BF16, kind="ExternalInput").ap()
    wq = nc.dram_tensor("wq", [D, DG], BF16, kind="ExternalInput").ap()
    wk = nc.dram_tensor("wk", [D, DG], BF16, kind="ExternalInput").ap()
    wv = nc.dram_tensor("wv", [D, DG], BF16, kind="ExternalInput").ap()
    wo = nc.dram_tensor("wo", [DG, D], BF16, kind="ExternalInput").ap()
    out = nc.dram_tensor("out", [S, D], BF16, kind="ExternalOutput").ap()

    with tile.TileContext(nc) as tc:
        with (
            tc.tile_pool(name="wpool", bufs=1) as wpool,
            tc.tile_pool(name="xin", bufs=1) as xin,
            tc.tile_pool(name="proj", bufs=1) as proj,
            tc.tile_pool(name="expp", bufs=22) as expp,
            tc.tile_pool(name="nrm", bufs=6) as nrm,
            tc.tile_pool(name="osbp", bufs=2) as osbp,
        ):
            # ---- ACT warmup: force the Exp table load at t=0 --------------
            wu_in = wpool.tile([128, 16], F32, tag="wu", name="wu_in")
            nc.vector.memset(wu_in, 0.0)
            wu_out = wpool.tile([128, 16], BF16, tag="wuo", name="wu_out")
            nc.scalar.activation(
                out=wu_out, in_=wu_in,
                func=mybir.ActivationFunctionType.Exp, scale=1.0,
            )

            # ---- constants + weight tiles ---------------------------------
            wk_sb = [wpool.tile([128, DG], BF16, tag=f"wk{e}", name=f"wk{e}")
                     for e in range(ET)]
            wq_sb = [wpool.tile([128, DG], BF16, tag=f"wq{e}", name=f"wq{e}")
                     for e in range(ET)]
            wv_sb = [wpool.tile([128, DG], BF16, tag=f"wv{e}", name=f"wv{e}")
                     for e in range(ET)]
            wo_sb = [wpool.tile([128, D], BF16, tag=f"wo{p}", name=f"wo{p}")
                     for p in range(2)]

            ident_f = wpool.tile([128, 128], F32, tag="ident_f", name="ident_f")
            make_identity(nc, ident_f)
            ident = wpool.tile([128, 128], BF16, tag="ident", name="ident")
            nc.vector.tensor_copy(ident, ident_f)

            # ---- persistent activation tiles ------------------------------
            kt_sb = [proj.tile([128, S], BF16, tag=f"kt{p}", name=f"kt{p}")
                     for p in range(2)]
            qt_sb = [proj.tile([128, S], BF16, tag=f"qt{p}", name=f"qt{p}")
                     for p in range(2)]
            v_sb = proj.tile([128, JT, HL, DK + 1], BF16, tag="v", name="v_sb")
            nc.vector.memset(v_sb[:, :, :, DK:DK + 1], 1.0)
            outt_sb = [proj.tile([128, S], BF16, tag=f"ot{p}", name=f"outt{p}")
                       for p in range(2)]

            # ---- DMA emission (3 queues round-robin) ----------------------
            # Order tuned so exp(c0) can start ~9.5us in: wk, xk[c0], wq,
            # xq[c0], xk[c1..c3] (keeps exp c0 fed), xv[c0], wv, then the
            # rest column-interleaved, wo last.
            queues = [nc.sync, nc.gpsimd]
            rr = [0]

            def dq(dst, src):
                queues[rr[0] % 2].dma_start(dst, src)
                rr[0] += 1

            xk_t = [xin.tile([128, S], BF16, tag=f"xk{e}", name=f"xk{e}")
                    for e in range(ET)]
            xq_t = [xin.tile([128, S], BF16, tag=f"xq{e}", name=f"xq{e}")
                    for e in range(ET)]
            xv_t = [xin.tile([128, S], BF16, tag=f"xv{e}", name=f"xv{e}")
                    for e in range(ET)]

            def dx(ts, dram, c):
                sl = slice(c * 512, (c + 1) * 512)
                for e in range(ET):
                    dq(ts[e][:, sl], dram[e * 128:(e + 1) * 128, sl])

            for e in range(ET):
                dq(wk_sb[e], wk[e * 128:(e + 1) * 128, :])
            dx(xk_t, xk, 0)
            for e in range(ET):
                dq(wq_sb[e], wq[e * 128:(e + 1) * 128, :])
            dx(xq_t, xq, 0)
            dx(xk_t, xk, 1)
            dx(xk_t, xk, 2)
            dx(xk_t, xk, 3)
            dx(xq_t, xq, 1)
            for e in range(ET):
                dq(wv_sb[e], wv[e * 128:(e + 1) * 128, :])
            dx(xv_t, xv, 0)
            dx(xv_t, xv, 1)
            dx(xq_t, xq, 2)
            dx(xv_t, xv, 2)
            dx(xv_t, xv, 3)
            dx(xq_t, xq, 3)
            for p in range(2):
                dq(wo_sb[p], wo[p * 128:(p + 1) * 128, :])

            # ---- phase A1: just enough for exp(c0) to start ---------------
            # The pool boundary below is a full barrier for successor pools,
            # so ps_a1 holds ONLY K-p0 (all S) + Q-p0-c0: it drains as soon
            # as xk + xq[c0] land (~22us) instead of after all of phase A.
            with tc.tile_pool(name="ps_a1", bufs=1, space="PSUM") as ps_a1:
                for c in range(QC):
                    csl = slice(c * 512, (c + 1) * 512)
                    acc = ps_a1.tile([128, 512], F32, tag="paq", bufs=4,
                                     name=f"ak0{c}")
                    for e in range(ET):
                        nc.tensor.matmul(
                            acc, wk_sb[e][:, 0:128], xk_t[e][:, csl],
                            start=(e == 0), stop=(e == ET - 1),
                        )
                    nc.vector.tensor_copy(kt_sb[0][:, csl], acc)
                accq = ps_a1.tile([128, 512], F32, tag="paq", bufs=4,
                                  name="aq00")
                for e in range(ET):
                    nc.tensor.matmul(
                        accq, wq_sb[e][:, 0:128], xq_t[e][:, 0:512],
                        start=(e == 0), stop=(e == ET - 1),
                    )
                nc.vector.tensor_copy(qt_sb[0][:, 0:512], accq)

            # ---- phase B: attention + output projection -------------------
            psacc = {}

            def tail(p, c, ex_tiles):
                """AV waves + normalize + transpose (+ Wo when p==1)."""
                ps_acc = psacc["p"]
                hA, hB = 2 * p, 2 * p + 1
                for k in range(4):          # one 128-query tile per wave
                    qsl = slice(k * 128, (k + 1) * 128)
                    accA = ps_acc.tile([128, DK + 1], F32, tag="acc",
                                       name=f"avA{p}{c}{k}")
                    accB = ps_acc.tile([128, DK + 1], F32, tag="acc",
                                       name=f"avB{p}{c}{k}")
                    for jt in range(JT):
                        nc.tensor.matmul(
                            accA, ex_tiles[jt][:, 0, qsl], v_sb[:, jt, hA, :],
                            start=(jt == 0), stop=(jt == JT - 1),
                        )
                        nc.tensor.matmul(
                            accB, ex_tiles[jt][:, 1, qsl], v_sb[:, jt, hB, :],
                            start=(jt == 0), stop=(jt == JT - 1),
                        )
                    recA = nrm.tile([128, 1], F32, tag="rec", name=f"rA{p}{c}{k}")
                    recB = nrm.tile([128, 1], F32, tag="rec", name=f"rB{p}{c}{k}")
                    nc.vector.reciprocal(recA, accA[:, DK:DK + 1])
                    nc.vector.reciprocal(recB, accB[:, DK:DK + 1])
                    nt = nrm.tile([128, 2, DK], BF16, tag="nt", name=f"nt{p}{c}{k}")
                    nc.vector.tensor_scalar(
                        nt[:, 0, :], accA[:, 0:DK], recA, None,
                        mybir.AluOpType.mult)
                    nc.vector.tensor_scalar(
                        nt[:, 1, :], accB[:, 0:DK], recB, None,
                        mybir.AluOpType.mult)
                    pt = ps_acc.tile([128, 128], BF16, tag="acc",
                                     name=f"pt{p}{c}{k}")
                    nc.tensor.transpose(pt, nt.rearrange("q h d -> q (h d)"),
                                        ident)
                    qg = c * 4 + k
                    nc.vector.tensor_copy(
                        outt_sb[p][:, qg * 128:(qg + 1) * 128], pt)
                if p == 1:
                    for k in range(4):
                        qg = c * 4 + k
                        osb = osbp.tile([128, D], BF16, tag="osb",
                                        name=f"osb{c}{k}")
                        for ch in range(2):
                            chsl = slice(ch * 512, (ch + 1) * 512)
                            acc = ps_acc.tile([128, 512], F32, tag="acc",
                                              name=f"po{c}{k}{ch}")
                            nc.tensor.matmul(
                                acc, outt_sb[0][:, qg * 128:(qg + 1) * 128],
                                wo_sb[0][:, chsl], start=True, stop=False)
                            nc.tensor.matmul(
                                acc, outt_sb[1][:, qg * 128:(qg + 1) * 128],
                                wo_sb[1][:, chsl], start=False, stop=True)
                            nc.vector.tensor_copy(osb[:, chsl], acc)
                        nc.sync.dma_start(
                            out[qg * 128:(qg + 1) * 128, :], osb)

            # ps_sc (banks 0-3, after ps_a1's barrier) runs the score ring
            # while ps_a2 (banks 4-7) finishes the REST of phase A under the
            # first two chunks' exp windows; ps_a2 then closes and ps_acc
            # (AV/transpose/Wo ring) takes over its banks.
            sc_cm = tc.tile_pool(name="ps_sc", bufs=2, space="PSUM")
            a2_cm = tc.tile_pool(name="ps_a2", bufs=4, space="PSUM")
            ps_sc = sc_cm.__enter__()
            ps_a2 = a2_cm.__enter__()

            def a2_kq(nm, p, cs):
                w_sb, x_t, dst = {
                    "k": (wk_sb, xk_t, kt_sb), "q": (wq_sb, xq_t, qt_sb),
                }[nm]
                for c in cs:
                    csl = slice(c * 512, (c + 1) * 512)
                    acc = ps_a2.tile([128, 512], F32, tag="pal",
                                     name=f"a{nm}{p}{c}")
                    for e in range(ET):
                        nc.tensor.matmul(
                            acc, w_sb[e][:, p * 128:(p + 1) * 128],
                            x_t[e][:, csl],
                            start=(e == 0), stop=(e == ET - 1))
                    nc.vector.tensor_copy(dst[p][:, csl], acc)

            def a2_v(jts):
                for jt in jts:
                    jsl = slice(jt * 128, (jt + 1) * 128)
                    accv = ps_a2.tile([128, DG], F32, tag="pal",
                                      name=f"av_{jt}")
                    for e in range(ET):
                        nc.tensor.matmul(
                            accv, xv_t[e][:, jsl], wv_sb[e],
                            start=(e == 0), stop=(e == ET - 1))
                    nc.vector.tensor_copy(
                        v_sb[:, jt, :, 0:DK],
                        accv.rearrange("j (h d) -> j h d", h=HL))

            a2_kq("q", 0, [1])
            a2_kq("k", 1, range(QC))

            pend = []
            for i in range(8):
                p, c = i // 4, i % 4
                csl = slice(c * 512, (c + 1) * 512)
                ex_tiles = []
                for jt in range(JT):
                    jsl = slice(jt * 128, (jt + 1) * 128)
                    sc = ps_sc.tile([128, 2, 512], F32, tag="sc",
                                    name=f"sc{p}{c}{jt}")
                    nc.tensor.matmul(
                        sc[:, 0, :], kt_sb[p][0:64, jsl],
                        qt_sb[p][0:64, csl],
                        start=True, stop=True, tile_position=(0, 0))
                    nc.tensor.matmul(
                        sc[:, 1, :], kt_sb[p][64:128, jsl],
                        qt_sb[p][64:128, csl],
                        start=True, stop=True, tile_position=(64, 0))
                    ex = expp.tile([128, 2, 512], BF16, tag="ex",
                                   name=f"ex{p}{c}{jt}")
                    nc.scalar.activation(
                        out=ex, in_=sc,
                        func=mybir.ActivationFunctionType.Exp,
                        scale=SCALE)
                    ex_tiles.append(ex)
                pend.append((p, c, ex_tiles))
                if i == 0:
                    a2_v(range(0, 8))
                elif i == 1:
                    a2_v(range(8, 16))
                    a2_kq("q", 0, [2, 3])
                    a2_kq("q", 1, range(QC))
                    a2_cm.__exit__(None, None, None)
                    acc_cm = tc.tile_pool(name="ps_acc", bufs=4,
                                          space="PSUM")
                    psacc["p"] = acc_cm.__enter__()
                if "p" in psacc:
                    while len(pend) > 1:
                        tail(*pend.pop(0))
            while pend:
                tail(*pend.pop(0))
            acc_cm.__exit__(None, None, None)
            sc_cm.__exit__(None, None, None)

    nc.compile()
    return nc


_NC = None


def _get_nc():
    global _NC
    if _NC is None:
        _NC = _build_nc()
    return _NC


def make_in_maps(query, key, value, Wq, Wk, Wv, Wo):
    bf = ml_dtypes.bfloat16
    xqT = [np.ascontiguousarray(np.asarray(query[b], dtype=np.float32).T.astype(bf))
           for b in range(B)]
    xkT = [np.ascontiguousarray(np.asarray(key[b], dtype=np.float32).T.astype(bf))
           for b in range(B)]
    xvT = [np.ascontiguousarray(np.asarray(value[b], dtype=np.float32).T.astype(bf))
           for b in range(B)]
    Wq = np.asarray(Wq, dtype=np.float32)
    Wk = np.asarray(Wk, dtype=np.float32)
    Wv = np.asarray(Wv, dtype=np.float32)
    Wo = np.asarray(Wo, dtype=np.float32)

    in_maps = []
    for core in range(8):
        b, g = divmod(core, 4)
        sl = slice(g * DG, (g + 1) * DG)
        in_maps.append({
            "xq": xqT[b],
            "xk": xkT[b],
            "xv": xvT[b],
            "wq": np.ascontiguousarray(Wq[:, sl].astype(bf)),
            "wk": np.ascontiguousarray(Wk[:, sl].astype(bf)),
            "wv": np.ascontiguousarray(Wv[:, sl].astype(bf)),
            "wo": np.ascontiguousarray(Wo[sl, :].astype(bf)),
        })
    return in_maps


def combine_results(results):
    out = np.zeros((B, S, D), dtype=np.float32)
    for core in range(8):
        out[core // 4] += results[core]["out"].astype(np.float32)
    return out


def kernel(query, key, value, Wq, Wk, Wv, Wo, _trace=False):
    from concourse import bass_utils

    nc = _get_nc()
    in_maps = make_in_maps(query, key, value, Wq, Wk, Wv, Wo)
    r = bass_utils.run_bass_kernel_spmd(
        nc, in_maps, core_ids=list(range(8)), trace=_trace
    )
    kernel.last_results = r
    return combine_results(r.results)


# revision 25
# speedup vs baseline: 1.5128x; 1.0408x over previous
"""Trainium2 Bass kernel for 16-head MultiHeadAttention (B=2, S=2048, D=1024, f32).

Sharding: 8 cores = 2 (batch) x 4 (head groups of 4 heads).
Each core gets col-shards of Wq/Wk/Wv ([1024,256]) + a row-shard of Wo
([256,1024]), computes a full [2048,1024] partial output; the host sums the
8 partials (4 per batch element) into [2,2048,1024].

All device data is bf16 (f32 accumulation in PSUM); the host converts inputs
and upconverts the bf16 partial outputs. rel-err budget 2e-2 >> bf16 noise.

On-device pipeline (per core):
  KT/QT = W^T @ x^T                    -> [128(=2 heads x 64), 2048] bf16
  V     = xv^T-tiles @ Wv directly     -> v_sb[j, jt, head, 65] (65th col = 1
          so the AV matmul emits softmax denominators for free)
  sT    = KT_h^T-slice @ QT_h-slice, two heads row-packed in the PE array
          via tile_position (0,0)/(64,0)          [128 j, 2, 512 q] PSUM f32
  expT  = exp(0.125 * sT) on ACT -> bf16 SBUF (ACT is the pacing engine:
          128 instrs x ~1.15us; everything else hides behind it)
  AV    : transposed orientation: lhsT = expT[j, q-slice], rhs = V_aug[j, 65]
          -> out[q, 65] PSUM, full 128x128 PE occupancy (2x fewer cycles than
          the dk-partition orientation). Column 64 = softmax denominator.
  norm  : per-partition reciprocal + tensor_scalar multiply (DVE)
  outT  : PE-transpose of normalized out back to [dg, q] for the Wo matmul
  out  += outT_p^T @ Wo_p accumulated over BOTH head pairs (K=256) -> one
          [2048, 1024] bf16 partial per core (half the output traffic).
"""

import sys

import numpy as np

if "/opt/trn_rl_repo" not in sys.path:
    sys.path.insert(0, "/opt/trn_rl_repo")

import ml_dtypes

import concourse.bacc as bacc
import concourse.mybir as mybir
import concourse.tile as tile
from concourse.masks import make_identity

F32 = mybir.dt.float32
BF16 = mybir.dt.bfloat16

B, S, D, H = 2, 2048, 1024, 16
DK = D // H          # 64
HL = 4               # heads per core
DG = HL * DK         # 256
SCALE = 0.125        # 1/sqrt(DK)

ET = D // 128        # 8 e-tiles (contraction over D)
JT = S // 128        # 16 j-tiles (keys)
QC = S // 512        # 4 q-chunks


def _build_nc():
    nc = bacc.Bacc("TRN2", target_bir_lowering=False, debug=False)

    xq = nc.dram_tensor("xq", [D, S], BF16, kind="ExternalInput").ap()
    xk = nc.dram_tensor("xk", [D, S], BF16, kind="ExternalInput").ap()
    xv = nc.dram_tensor("xv", [D, S], BF16, kind="ExternalInput").ap()
    wq = nc.dram_tensor("wq", [D, DG], BF16, kind="ExternalInput").ap()
    wk = nc.dram_tensor("wk", [D, DG], BF16, kind="ExternalInput").ap()
    wv = nc.dram_tensor("wv", [D, DG], BF16, kind="ExternalInput").ap()
    wo = nc.dram_tensor("wo", [DG, D], BF16, kind="ExternalInput").ap()
    out = nc.dram_tensor("out", [S, D], BF16, kind="ExternalOutput").ap()

    with tile.TileContext(nc) as tc:
        with (
            tc.tile_pool(name="wpool", bufs=1) as wpool,
            tc.tile_pool(name="xin", bufs=1) as xin,
            tc.tile_pool(name="proj", bufs=1) as proj,
            tc.tile_pool(name="expp", bufs=22) as expp,
            tc.tile_pool(name="nrm", bufs=6) as nrm,
            tc.tile_pool(name="osbp", bufs=2) as osbp,
        ):
            # ---- ACT warmup: force the Exp table load at t=0 --------------
            wu_in = wpool.tile([128, 16], F32, tag="wu", name="wu_in")
            nc.vector.memset(wu_in, 0.0)
            wu_out = wpool.tile([128, 16], BF16, tag="wuo", name="wu_out")
            nc.scalar.activation(
                out=wu_out, in_=wu_in,
                func=mybir.ActivationFunctionType.Exp, scale=1.0,
            )

            # ---- constants + weight tiles ---------------------------------
            wk_sb = [wpool.tile([128, DG], BF16, tag=f"wk{e}", name=f"wk{e}")
                     for e in range(ET)]
            wq_sb = [wpool.tile([128, DG], BF16, tag=f"wq{e}", name=f"wq{e}")
                     for e in range(ET)]
            wv_sb = [wpool.tile([128, DG], BF16, tag=f"wv{e}", name=f"wv{e}")
                     for e in range(ET)]
            wo_sb = [wpool.tile([128, D], BF16, tag=f"wo{p}", name=f"wo{p}")
                     for p in range(2)]

            ident_f = wpool.tile([128, 128], F32, tag="ident_f", name="ident_f")
            make_identity(nc, ident_f)
            ident = wpool.tile([128, 128], BF16, tag="ident", name="ident")
            nc.vector.tensor_copy(ident, ident_f)

            # ---- persistent activation tiles ------------------------------
            kt_sb = [proj.tile([128, S], BF16, tag=f"kt{p}", name=f"kt{p}")
                     for p in range(2)]
            qt_sb = [proj.tile([128, S], BF16, tag=f"qt{p}", name=f"qt{p}")
                     for p in range(2)]
            v_sb = proj.tile([128, JT, HL, DK + 1], BF16, tag="v", name="v_sb")
            nc.vector.memset(v_sb[:, :, :, DK:DK + 1], 1.0)
            outt_sb = [proj.tile([128, S], BF16, tag=f"ot{p}", name=f"outt{p}")
                       for p in range(2)]

            # ---- DMA emission (3 queues round-robin) ----------------------
            # Order tuned so exp(c0) can start ~9.5us in: wk, xk[c0], wq,
            # xq[c0], xk[c1..c3] (keeps exp c0 fed), xv[c0], wv, then the
            # rest column-interleaved, wo last.
            queues = [nc.sync, nc.gpsimd]
            rr = [0]

            def dq(dst, src):
                queues[rr[0] % 2].dma_start(dst, src)
                rr[0] += 1

            xk_t = [xin.tile([128, S], BF16, tag=f"xk{e}", name=f"xk{e}")
                    for e in range(ET)]
            xq_t = [xin.tile([128, S], BF16, tag=f"xq{e}", name=f"xq{e}")
                    for e in range(ET)]
            xv_t = [xin.tile([128, S], BF16, tag=f"xv{e}", name=f"xv{e}")
                    for e in range(ET)]

            def dx(ts, dram, c):
                sl = slice(c * 512, (c + 1) * 512)
                for e in range(ET):
                    dq(ts[e][:, sl], dram[e * 128:(e + 1) * 128, sl])

            for e in range(ET):
                dq(wk_sb[e], wk[e * 128:(e + 1) * 128, :])
            dx(xk_t, xk, 0)
            dx(xk_t, xk, 1)
            for e in range(ET):
                dq(wq_sb[e], wq[e * 128:(e + 1) * 128, :])
            dx(xq_t, xq, 0)
            dx(xk_t, xk, 2)
            dx(xk_t, xk, 3)
            dx(xq_t, xq, 1)
            for e in range(ET):
                dq(wv_sb[e], wv[e * 128:(e + 1) * 128, :])
            dx(xv_t, xv, 0)
            dx(xv_t, xv, 1)
            dx(xv_t, xv, 2)
            dx(xv_t, xv, 3)
            dx(xq_t, xq, 2)
            dx(xq_t, xq, 3)
            for p in range(2):
                dq(wo_sb[p], wo[p * 128:(p + 1) * 128, :])

            # ---- phase A1: just enough for exp(c0) to start ---------------
            # The pool boundary below is a full barrier for successor pools,
            # so ps_a1 holds ONLY K-p0 (all S) + Q-p0-c0: it drains as soon
            # as xk + xq[c0] land (~22us) instead of after all of phase A.
            with tc.tile_pool(name="ps_a1", bufs=1, space="PSUM") as ps_a1:
                for nm, c in (("k", 0), ("k", 1), ("q", 0), ("k", 2), ("k", 3)):
                    w_sb, x_t, dst = {
                        "k": (wk_sb, xk_t, kt_sb), "q": (wq_sb, xq_t, qt_sb),
                    }[nm]
                    csl = slice(c * 512, (c + 1) * 512)
                    acc = ps_a1.tile([128, 512], F32, tag="paq", bufs=4,
                                     name=f"a{nm}0{c}")
                    for e in range(ET):
                        nc.tensor.matmul(
                            acc, w_sb[e][:, 0:128], x_t[e][:, csl],
                            start=(e == 0), stop=(e == ET - 1),
                        )
                    nc.vector.tensor_copy(dst[0][:, csl], acc)

            # ---- phase B: attention + output projection -------------------
            psacc = {}

            def tail(p, c, ex_tiles):
                """AV waves + normalize + transpose (+ Wo when p==1)."""
                ps_acc = psacc["p"]
                hA, hB = 2 * p, 2 * p + 1
                for k in range(4):          # one 128-query tile per wave
                    qsl = slice(k * 128, (k + 1) * 128)
                    accA = ps_acc.tile([128, DK + 1], F32, tag="acc",
                                       name=f"avA{p}{c}{k}")
                    accB = ps_acc.tile([128, DK + 1], F32, tag="acc",
                                       name=f"avB{p}{c}{k}")
                    for jt in range(JT):
                        nc.tensor.matmul(
                            accA, ex_tiles[jt][:, 0, qsl], v_sb[:, jt, hA, :],
                            start=(jt == 0), stop=(jt == JT - 1),
                        )
                        nc.tensor.matmul(
                            accB, ex_tiles[jt][:, 1, qsl], v_sb[:, jt, hB, :],
                            start=(jt == 0), stop=(jt == JT - 1),
                        )
                    recA = nrm.tile([128, 1], F32, tag="rec", name=f"rA{p}{c}{k}")
                    recB = nrm.tile([128, 1], F32, tag="rec", name=f"rB{p}{c}{k}")
                    nc.vector.reciprocal(recA, accA[:, DK:DK + 1])
                    nc.vector.reciprocal(recB, accB[:, DK:DK + 1])
                    nt = nrm.tile([128, 2, DK], BF16, tag="nt", name=f"nt{p}{c}{k}")
                    nc.vector.tensor_scalar(
                        nt[:, 0, :], accA[:, 0:DK], recA, None,
                        mybir.AluOpType.mult)
                    nc.vector.tensor_scalar(
                        nt[:, 1, :], accB[:, 0:DK], recB, None,
                        mybir.AluOpType.mult)
                    pt = ps_acc.tile([128, 128], BF16, tag="acc",
                                     name=f"pt{p}{c}{k}")
                    nc.tensor.transpose(pt, nt.rearrange("q h d -> q (h d)"),
                                        ident)
                    qg = c * 4 + k
                    nc.vector.tensor_copy(
                        outt_sb[p][:, qg * 128:(qg + 1) * 128], pt)
                if p == 1:
                    for k in range(4):
                        qg = c * 4 + k
                        osb = osbp.tile([128, D], BF16, tag="osb",
                                        name=f"osb{c}{k}")
                        for ch in range(2):
                            chsl = slice(ch * 512, (ch + 1) * 512)
                            acc = ps_acc.tile([128, 512], F32, tag="acc",
                                              name=f"po{c}{k}{ch}")
                            nc.tensor.matmul(
                                acc, outt_sb[0][:, qg * 128:(qg + 1) * 128],
                                wo_sb[0][:, chsl], start=True, stop=False)
                            nc.tensor.matmul(
                                acc, outt_sb[1][:, qg * 128:(qg + 1) * 128],
                                wo_sb[1][:, chsl], start=False, stop=True)
                            nc.vector.tensor_copy(osb[:, chsl], acc)
                        nc.sync.dma_start(
                            out[qg * 128:(qg + 1) * 128, :], osb)

            # ps_sc (banks 0-3, after ps_a1's barrier) runs the score ring
            # while ps_a2 (banks 4-7) finishes the REST of phase A under the
            # first two chunks' exp windows; ps_a2 then closes and ps_acc
            # (AV/transpose/Wo ring) takes over its banks.
            sc_cm = tc.tile_pool(name="ps_sc", bufs=2, space="PSUM")
            a2_cm = tc.tile_pool(name="ps_a2", bufs=4, space="PSUM")
            ps_sc = sc_cm.__enter__()
            ps_a2 = a2_cm.__enter__()

            def a2_kq(nm, p, cs):
                w_sb, x_t, dst = {
                    "k": (wk_sb, xk_t, kt_sb), "q": (wq_sb, xq_t, qt_sb),
                }[nm]
                for c in cs:
                    csl = slice(c * 512, (c + 1) * 512)
                    acc = ps_a2.tile([128, 512], F32, tag="pal",
                                     name=f"a{nm}{p}{c}")
                    for e in range(ET):
                        nc.tensor.matmul(
                            acc, w_sb[e][:, p * 128:(p + 1) * 128],
                            x_t[e][:, csl],
                            start=(e == 0), stop=(e == ET - 1))
                    nc.vector.tensor_copy(dst[p][:, csl], acc)

            def a2_v(jts):
                for jt in jts:
                    jsl = slice(jt * 128, (jt + 1) * 128)
                    accv = ps_a2.tile([128, DG], F32, tag="pal",
                                      name=f"av_{jt}")
                    for e in range(ET):
                        nc.tensor.matmul(
                            accv, xv_t[e][:, jsl], wv_sb[e],
                            start=(e == 0), stop=(e == ET - 1))
                    nc.vector.tensor_copy(
                        v_sb[:, jt, :, 0:DK],
                        accv.rearrange("j (h d) -> j h d", h=HL))

            def b_kq(nm, p, cs):
                """Late Q projections on the phase-B acc ring."""
                w_sb, x_t, dst = {
                    "k": (wk_sb, xk_t, kt_sb), "q": (wq_sb, xq_t, qt_sb),
                }[nm]
                for c in cs:
                    csl = slice(c * 512, (c + 1) * 512)
                    acc = psacc["p"].tile([128, 512], F32, tag="acc",
                                          name=f"b{nm}{p}{c}")
                    for e in range(ET):
                        nc.tensor.matmul(
                            acc, w_sb[e][:, p * 128:(p + 1) * 128],
                            x_t[e][:, csl],
                            start=(e == 0), stop=(e == ET - 1))
                    nc.vector.tensor_copy(dst[p][:, csl], acc)

            pend = []
            for i in range(8):
                p, c = i // 4, i % 4
                csl = slice(c * 512, (c + 1) * 512)
                ex_tiles = []
                for jt in range(JT):
                    jsl = slice(jt * 128, (jt + 1) * 128)
                    sc = ps_sc.tile([128, 2, 512], F32, tag="sc",
                                    name=f"sc{p}{c}{jt}")
                    nc.tensor.matmul(
                        sc[:, 0, :], kt_sb[p][0:64, jsl],
                        qt_sb[p][0:64, csl],
                        start=True, stop=True, tile_position=(0, 0))
                    nc.tensor.matmul(
                        sc[:, 1, :], kt_sb[p][64:128, jsl],
                        qt_sb[p][64:128, csl],
                        start=True, stop=True, tile_position=(64, 0))
                    ex = expp.tile([128, 2, 512], BF16, tag="ex",
                                   name=f"ex{p}{c}{jt}")
                    nc.scalar.activation(
                        out=ex, in_=sc,
                        func=mybir.ActivationFunctionType.Exp,
                        scale=SCALE)
                    ex_tiles.append(ex)
                pend.append((p, c, ex_tiles))
                if i == 0:
                    a2_kq("q", 0, [1])
                    a2_kq("k", 1, range(QC))
                    a2_v(range(0, 8))
                elif i == 1:
                    a2_v(range(8, 16))
                    a2_cm.__exit__(None, None, None)
                    acc_cm = tc.tile_pool(name="ps_acc", bufs=4,
                                          space="PSUM")
                    psacc["p"] = acc_cm.__enter__()
                    b_kq("q", 0, [2])
                elif i == 2:
                    b_kq("q", 0, [3])
                    b_kq("q", 1, [0])
                elif i == 3:
                    b_kq("q", 1, [1, 2])
                elif i == 4:
                    b_kq("q", 1, [3])
                if "p" in psacc:
                    while len(pend) > 1:
                        tail(*pend.pop(0))
            while pend:
                tail(*pend.pop(0))
            acc_cm.__exit__(None, None, None)
            sc_cm.__exit__(None, None, None)

    nc.compile()
    return nc


_NC = None


def _get_nc():
    global _NC
    if _NC is None:
        _NC = _build_nc()
    return _NC


def make_in_maps(query, key, value, Wq, Wk, Wv, Wo):
    bf = ml_dtypes.bfloat16
    xqT = [np.ascontiguousarray(np.asarray(query[b], dtype=np.float32).T.astype(bf))
           for b in range(B)]
    xkT = [np.ascontiguousarray(np.asarray(key[b], dtype=np.float32).T.astype(bf))
           for b in range(B)]
    xvT = [np.ascontiguousarray(np.asarray(value[b], dtype=np.float32).T.astype(bf))
           for b in range(B)]
    Wq = np.asarray(Wq, dtype=np.float32)
    Wk = np.asarray(Wk, dtype=np.float32)
    Wv = np.asarray(Wv, dtype=np.float32)
    Wo = np.asarray(Wo, dtype=np.float32)

    in_maps = []
    for core in range(8):
        b, g = divmod(core, 4)
        sl = slice(g * DG, (g + 1) * DG)
        in_maps.append({
            "xq": xqT[b],
            "xk": xkT[b],
            "xv": xvT[b],
            "wq": np.ascontiguousarray(Wq[:, sl].astype(bf)),
            "wk": np.ascontiguousarray(Wk[:, sl].astype(bf)),
            "wv": np.ascontiguousarray(Wv[:, sl].astype(bf)),
            "wo": np.ascontiguousarray(Wo[sl, :].astype(bf)),
        })
    return in_maps


def combine_results(results):
    out = np.zeros((B, S, D), dtype=np.float32)
    for core in range(8):
        out[core // 4] += results[core]["out"].astype(np.float32)
    return out


def kernel(query, key, value, Wq, Wk, Wv, Wo, _trace=False):
    from concourse import bass_utils

    nc = _get_nc()
    in_maps = make_in_maps(query, key, value, Wq, Wk, Wv, Wo)
    r = bass_utils.run_bass_kernel_spmd(
        nc, in_maps, core_ids=list(range(8)), trace=_trace
    )
    kernel.last_results = r
    return combine_results(r.results)


# revision 29
# speedup vs baseline: 1.8487x; 1.2220x over previous
"""Trainium2 Bass kernel for 16-head MultiHeadAttention (B=2, S=2048, D=1024, f32).

Sharding: 8 cores = 2 (batch) x 4 (head groups of 4 heads).
Each core gets col-shards of Wq/Wk/Wv ([1024,256]) + a row-shard of Wo
([256,1024]), computes a full [2048,1024] partial output; the host sums the
8 partials (4 per batch element) into [2,2048,1024].

All device data is bf16 (f32 accumulation in PSUM); the host converts inputs
and upconverts the bf16 partial outputs. rel-err budget 2e-2 >> bf16 noise.

On-device pipeline (per core):
  KT/QT = W^T @ x^T                    -> [128(=2 heads x 64), 2048] bf16
  V     = xv^T-tiles @ Wv directly     -> v_sb[j, jt, head, 65] (65th col = 1
          so the AV matmul emits softmax denominators for free)
  sT    = KT_h^T-slice @ QT_h-slice, two heads row-packed in the PE array
          via tile_position (0,0)/(64,0)          [128 j, 2, 512 q] PSUM f32
  expT  = exp(0.125 * sT) on ACT -> bf16 SBUF (ACT is the pacing engine:
          128 instrs x ~1.15us; everything else hides behind it)
  AV    : transposed orientation: lhsT = expT[j, q-slice], rhs = V_aug[j, 65]
          -> out[q, 65] PSUM, full 128x128 PE occupancy (2x fewer cycles than
          the dk-partition orientation). Column 64 = softmax denominator.
  norm  : per-partition reciprocal + tensor_scalar multiply (DVE)
  outT  : PE-transpose of normalized out back to [dg, q] for the Wo matmul
  out  += outT_p^T @ Wo_p accumulated over BOTH head pairs (K=256) -> one
          [2048, 1024] bf16 partial per core (half the output traffic).
"""

import sys

import numpy as np

if "/opt/trn_rl_repo" not in sys.path:
    sys.path.insert(0, "/opt/trn_rl_repo")

import ml_dtypes

import concourse.bacc as bacc
import concourse.mybir as mybir
import concourse.tile as tile
from concourse.masks import make_identity

F32 = mybir.dt.float32
BF16 = mybir.dt.bfloat16

B, S, D, H = 2, 2048, 1024, 16
DK = D // H          # 64
HL = 4               # heads per core
DG = HL * DK         # 256
SCALE = 0.125        # 1/sqrt(DK)

ET = D // 128        # 8 e-tiles (contraction over D)
JT = S // 128        # 16 j-tiles (keys)
QC = S // 512        # 4 q-chunks


def _build_nc():
    nc = bacc.Bacc("TRN2", target_bir_lowering=False, debug=False)

    xq = nc.dram_tensor("xq", [D, S], BF16, kind="ExternalInput").ap()
    xk = nc.dram_tensor("xk", [D, S], BF16, kind="ExternalInput").ap()
    xv = nc.dram_tensor("xv", [D, S], BF16, kind="ExternalInput").ap()
    wq = nc.dram_tensor("wq", [D, DG], BF16, kind="ExternalInput").ap()
    wk = nc.dram_tensor("wk", [D, DG], BF16, kind="ExternalInput").ap()
    wv = nc.dram_tensor("wv", [D, DG], BF16, kind="ExternalInput").ap()
    wo = nc.dram_tensor("wo", [DG, D], BF16, kind="ExternalInput").ap()
    out = nc.dram_tensor("out", [S, D], BF16, kind="ExternalOutput").ap()

    with tile.TileContext(nc) as tc:
        with (
            tc.tile_pool(name="wpool", bufs=1) as wpool,
            tc.tile_pool(name="xin", bufs=1) as xin,
            tc.tile_pool(name="proj", bufs=1) as proj,
            tc.tile_pool(name="expp", bufs=32) as expp,
            tc.tile_pool(name="nrm", bufs=6) as nrm,
            tc.tile_pool(name="osbp", bufs=2) as osbp,
        ):
            # ---- ACT warmup: force the Exp table load at t=0 --------------
            wu_in = wpool.tile([128, 16], F32, tag="wu", name="wu_in")
            nc.vector.memset(wu_in, 0.0)
            wu_out = wpool.tile([128, 16], BF16, tag="wuo", name="wu_out")
            nc.scalar.activation(
                out=wu_out, in_=wu_in,
                func=mybir.ActivationFunctionType.Exp, scale=1.0,
            )

            # ---- constants + weight tiles ---------------------------------
            wk_sb = [wpool.tile([128, DG], BF16, tag=f"wk{e}", name=f"wk{e}")
                     for e in range(ET)]
            wq_sb = [wpool.tile([128, DG], BF16, tag=f"wq{e}", name=f"wq{e}")
                     for e in range(ET)]
            wv_sb = [wpool.tile([128, DG], BF16, tag=f"wv{e}", name=f"wv{e}")
                     for e in range(ET)]
            wo_sb = [wpool.tile([128, D], BF16, tag=f"wo{p}", name=f"wo{p}")
                     for p in range(2)]

            ident_f = wpool.tile([128, 128], F32, tag="ident_f", name="ident_f")
            make_identity(nc, ident_f)
            ident = wpool.tile([128, 128], BF16, tag="ident", name="ident")
            nc.vector.tensor_copy(ident, ident_f)

            # ---- persistent activation tiles ------------------------------
            kt_sb = [proj.tile([128, S], BF16, tag=f"kt{p}", name=f"kt{p}")
                     for p in range(2)]
            qt_sb = [proj.tile([128, S], BF16, tag=f"qt{p}", name=f"qt{p}")
                     for p in range(2)]
            v_sb = proj.tile([128, JT, HL, DK + 1], BF16, tag="v", name="v_sb")
            nc.vector.memset(v_sb[:, :, :, DK:DK + 1], 1.0)
            outt_sb = [proj.tile([128, S], BF16, tag=f"ot{p}", name=f"outt{p}")
                       for p in range(2)]

            # ---- DMA emission (3 queues round-robin) ----------------------
            # Order tuned so exp(c0) can start ~9.5us in: wk, xk[c0], wq,
            # xq[c0], xk[c1..c3] (keeps exp c0 fed), xv[c0], wv, then the
            # rest column-interleaved, wo last.
            queues = [nc.sync, nc.gpsimd]
            rr = [0]

            def dq(dst, src):
                queues[rr[0] % 2].dma_start(dst, src)
                rr[0] += 1

            xk_t = [xin.tile([128, S], BF16, tag=f"xk{e}", name=f"xk{e}")
                    for e in range(ET)]
            xq_t = [xin.tile([128, S], BF16, tag=f"xq{e}", name=f"xq{e}")
                    for e in range(ET)]
            # xv reuses xk's SBUF slots: xk's last readers (K-p1) run before
            # xv's transfers reach the queue head, so this costs no time.
            xv_t = [xin.tile([128, S], BF16, tag=f"xk{e}", name=f"xv{e}")
                    for e in range(ET)]

            def dx(ts, dram, c):
                sl = slice(c * 512, (c + 1) * 512)
                for e in range(ET):
                    dq(ts[e][:, sl], dram[e * 128:(e + 1) * 128, sl])

            for e in range(ET):
                dq(wk_sb[e], wk[e * 128:(e + 1) * 128, :])
            dx(xk_t, xk, 0)
            dx(xk_t, xk, 1)
            for e in range(ET):
                dq(wq_sb[e], wq[e * 128:(e + 1) * 128, :])
            dx(xq_t, xq, 0)
            dx(xk_t, xk, 2)
            dx(xk_t, xk, 3)
            dx(xq_t, xq, 1)
            for e in range(ET):
                dq(wv_sb[e], wv[e * 128:(e + 1) * 128, :])
            dx(xv_t, xv, 0)
            dx(xv_t, xv, 1)
            dx(xv_t, xv, 2)
            dx(xv_t, xv, 3)
            dx(xq_t, xq, 2)
            dx(xq_t, xq, 3)
            for p in range(2):
                dq(wo_sb[p], wo[p * 128:(p + 1) * 128, :])

            # ---- phase A1: just enough for exp(c0) to start ---------------
            # The pool boundary below is a full barrier for successor pools,
            # so ps_a1 holds ONLY K-p0 (all S) + Q-p0-c0: it drains as soon
            # as xk + xq[c0] land (~22us) instead of after all of phase A.
            with tc.tile_pool(name="ps_a1", bufs=1, space="PSUM") as ps_a1:
                for nm, c in (("k", 0), ("q", 0)):
                    w_sb, x_t, dst = {
                        "k": (wk_sb, xk_t, kt_sb), "q": (wq_sb, xq_t, qt_sb),
                    }[nm]
                    csl = slice(c * 512, (c + 1) * 512)
                    acc = ps_a1.tile([128, 512], F32, tag="paq", bufs=2,
                                     name=f"a{nm}0{c}")
                    for e in range(ET):
                        nc.tensor.matmul(
                            acc, w_sb[e][:, 0:128], x_t[e][:, csl],
                            start=(e == 0), stop=(e == ET - 1),
                        )
                    nc.vector.tensor_copy(dst[0][:, csl], acc)

            # ---- phase B: attention + output projection -------------------
            psacc = {}

            def tail(p, c, ex_tiles):
                """AV waves + normalize + transpose (+ Wo when p==1)."""
                ps_acc = psacc["p"]
                hA, hB = 2 * p, 2 * p + 1
                for k in range(4):          # one 128-query tile per wave
                    qsl = slice(k * 128, (k + 1) * 128)
                    accA = ps_acc.tile([128, DK + 1], F32, tag="acc",
                                       name=f"avA{p}{c}{k}")
                    accB = ps_acc.tile([128, DK + 1], F32, tag="acc",
                                       name=f"avB{p}{c}{k}")
                    for jt in range(JT):
                        nc.tensor.matmul(
                            accA, ex_tiles[jt][:, 0, qsl], v_sb[:, jt, hA, :],
                            start=(jt == 0), stop=(jt == JT - 1),
                        )
                        nc.tensor.matmul(
                            accB, ex_tiles[jt][:, 1, qsl], v_sb[:, jt, hB, :],
                            start=(jt == 0), stop=(jt == JT - 1),
                        )
                    recA = nrm.tile([128, 1], F32, tag="rec", name=f"rA{p}{c}{k}")
                    recB = nrm.tile([128, 1], F32, tag="rec", name=f"rB{p}{c}{k}")
                    nc.vector.reciprocal(recA, accA[:, DK:DK + 1])
                    nc.vector.reciprocal(recB, accB[:, DK:DK + 1])
                    nt = nrm.tile([128, 2, DK], BF16, tag="nt", name=f"nt{p}{c}{k}")
                    nc.vector.tensor_scalar(
                        nt[:, 0, :], accA[:, 0:DK], recA, None,
                        mybir.AluOpType.mult)
                    nc.vector.tensor_scalar(
                        nt[:, 1, :], accB[:, 0:DK], recB, None,
                        mybir.AluOpType.mult)
                    pt = ps_acc.tile([128, 128], BF16, tag="acc",
                                     name=f"pt{p}{c}{k}")
                    nc.tensor.transpose(pt, nt.rearrange("q h d -> q (h d)"),
                                        ident)
                    qg = c * 4 + k
                    nc.vector.tensor_copy(
                        outt_sb[p][:, qg * 128:(qg + 1) * 128], pt)
                if p == 1:
                    for k in range(4):
                        qg = c * 4 + k
                        osb = osbp.tile([128, D], BF16, tag="osb",
                                        name=f"osb{c}{k}")
                        for ch in range(2):
                            chsl = slice(ch * 512, (ch + 1) * 512)
                            acc = ps_acc.tile([128, 512], F32, tag="acc",
                                              name=f"po{c}{k}{ch}")
                            nc.tensor.matmul(
                                acc, outt_sb[0][:, qg * 128:(qg + 1) * 128],
                                wo_sb[0][:, chsl], start=True, stop=False)
                            nc.tensor.matmul(
                                acc, outt_sb[1][:, qg * 128:(qg + 1) * 128],
                                wo_sb[1][:, chsl], start=False, stop=True)
                            nc.vector.tensor_copy(osb[:, chsl], acc)
                        nc.sync.dma_start(
                            out[qg * 128:(qg + 1) * 128, :], osb)

            # ps_sc (banks 0-3, after ps_a1's barrier) runs the score ring
            # while ps_a2 (banks 4-7) finishes the REST of phase A under the
            # first two chunks' exp windows; ps_a2 then closes and ps_acc
            # (AV/transpose/Wo ring) takes over its banks.
            sc_cm = tc.tile_pool(name="ps_sc", bufs=2, space="PSUM")
            a2_cm = tc.tile_pool(name="ps_a2", bufs=4, space="PSUM")
            ps_sc = sc_cm.__enter__()
            ps_a2 = a2_cm.__enter__()

            def a2_kq(nm, p, cs):
                w_sb, x_t, dst = {
                    "k": (wk_sb, xk_t, kt_sb), "q": (wq_sb, xq_t, qt_sb),
                }[nm]
                for c in cs:
                    csl = slice(c * 512, (c + 1) * 512)
                    acc = ps_a2.tile([128, 512], F32, tag="pal",
                                     name=f"a{nm}{p}{c}")
                    for e in range(ET):
                        nc.tensor.matmul(
                            acc, w_sb[e][:, p * 128:(p + 1) * 128],
                            x_t[e][:, csl],
                            start=(e == 0), stop=(e == ET - 1))
                    nc.vector.tensor_copy(dst[p][:, csl], acc)

            def a2_v(jts):
                for jt in jts:
                    jsl = slice(jt * 128, (jt + 1) * 128)
                    accv = ps_a2.tile([128, DG], F32, tag="pal",
                                      name=f"av_{jt}")
                    for e in range(ET):
                        nc.tensor.matmul(
                            accv, xv_t[e][:, jsl], wv_sb[e],
                            start=(e == 0), stop=(e == ET - 1))
                    nc.vector.tensor_copy(
                        v_sb[:, jt, :, 0:DK],
                        accv.rearrange("j (h d) -> j h d", h=HL))

            def b_kq(nm, p, cs):
                """Late Q projections on the phase-B acc ring."""
                w_sb, x_t, dst = {
                    "k": (wk_sb, xk_t, kt_sb), "q": (wq_sb, xq_t, qt_sb),
                }[nm]
                for c in cs:
                    csl = slice(c * 512, (c + 1) * 512)
                    acc = psacc["p"].tile([128, 512], F32, tag="acc",
                                          name=f"b{nm}{p}{c}")
                    for e in range(ET):
                        nc.tensor.matmul(
                            acc, w_sb[e][:, p * 128:(p + 1) * 128],
                            x_t[e][:, csl],
                            start=(e == 0), stop=(e == ET - 1))
                    nc.vector.tensor_copy(dst[p][:, csl], acc)

            pend = []
            for i in range(8):
                p, c = i // 4, i % 4
                csl = slice(c * 512, (c + 1) * 512)
                ex_tiles = []
                for jt in range(JT):
                    # K-p0 c1..c3 land just before the scores that read them
                    # (kt columns jt*128 onward), off the ps_a1 barrier path.
                    if i == 0 and jt in (4, 8, 12):
                        a2_kq("k", 0, [jt // 4])
                    jsl = slice(jt * 128, (jt + 1) * 128)
                    sc = ps_sc.tile([128, 2, 512], F32, tag="sc",
                                    name=f"sc{p}{c}{jt}")
                    nc.tensor.matmul(
                        sc[:, 0, :], kt_sb[p][0:64, jsl],
                        qt_sb[p][0:64, csl],
                        start=True, stop=True, tile_position=(0, 0))
                    nc.tensor.matmul(
                        sc[:, 1, :], kt_sb[p][64:128, jsl],
                        qt_sb[p][64:128, csl],
                        start=True, stop=True, tile_position=(64, 0))
                    ex = expp.tile([128, 2, 512], BF16, tag="ex",
                                   name=f"ex{p}{c}{jt}")
                    nc.scalar.activation(
                        out=ex, in_=sc,
                        func=mybir.ActivationFunctionType.Exp,
                        scale=SCALE)
                    ex_tiles.append(ex)
                pend.append((p, c, ex_tiles))
                if i == 0:
                    a2_kq("q", 0, [1])
                    a2_kq("k", 1, range(QC))
                    a2_v(range(0, 8))
                elif i == 1:
                    a2_v(range(8, 16))
                    a2_cm.__exit__(None, None, None)
                    acc_cm = tc.tile_pool(name="ps_acc", bufs=4,
                                          space="PSUM")
                    psacc["p"] = acc_cm.__enter__()
                    b_kq("q", 0, [2])
                elif i == 2:
                    b_kq("q", 0, [3])
                    b_kq("q", 1, [0])
                elif i == 3:
                    b_kq("q", 1, [1, 2])
                elif i == 4:
                    b_kq("q", 1, [3])
                if "p" in psacc:
                    while len(pend) > 1:
                        tail(*pend.pop(0))
            while pend:
                tail(*pend.pop(0))
            acc_cm.__exit__(None, None, None)
            sc_cm.__exit__(None, None, None)

    nc.compile()
    return nc


_NC = None


def _get_nc():
    global _NC
    if _NC is None:
        _NC = _build_nc()
    return _NC


def make_in_maps(query, key, value, Wq, Wk, Wv, Wo):
    bf = ml_dtypes.bfloat16
    xqT = [np.ascontiguousarray(np.asarray(query[b], dtype=np.float32).T.astype(bf))
           for b in range(B)]
    xkT = [np.ascontiguousarray(np.asarray(key[b], dtype=np.float32).T.astype(bf))
           for b in range(B)]
    xvT = [np.ascontiguousarray(np.asarray(value[b], dtype=np.float32).T.astype(bf))
           for b in range(B)]
    Wq = np.asarray(Wq, dtype=np.float32)
    Wk = np.asarray(Wk, dtype=np.float32)
    Wv = np.asarray(Wv, dtype=np.float32)
    Wo = np.asarray(Wo, dtype=np.float32)

    in_maps = []
    for core in range(8):
        b, g = divmod(core, 4)
        sl = slice(g * DG, (g + 1) * DG)
        in_maps.append({
            "xq": xqT[b],
            "xk": xkT[b],
            "xv": xvT[b],
            "wq": np.ascontiguousarray(Wq[:, sl].astype(bf)),
            "wk": np.ascontiguousarray(Wk[:, sl].astype(bf)),
            "wv": np.ascontiguousarray(Wv[:, sl].astype(bf)),
            "wo": np.ascontiguousarray(Wo[sl, :].astype(bf)),
        })
    return in_maps


def combine_results(results):
    out = np.zeros((B, S, D), dtype=np.float32)
    for core in range(8):
        out[core // 4] += results[core]["out"].astype(np.float32)
    return out


def kernel(query, key, value, Wq, Wk, Wv, Wo, _trace=False):
    from concourse import bass_utils

    nc = _get_nc()
    in_maps = make_in_maps(query, key, value, Wq, Wk, Wv, Wo)
    r = bass_utils.run_bass_kernel_spmd(
        nc, in_maps, core_ids=list(range(8)), trace=_trace
    )
    kernel.last_results = r
    return combine_results(r.results)
